# revision 1
# baseline (speedup 1.0000x reference)
"""BipartiteGCN Trainium2 kernel (8 NeuronCores, Bass/Tile).

Strategy: shard message DESTINATIONS across cores (authors 12500/core, active
papers 12500/core; papers >= 100000 never influence the output since all paper
indices are drawn from [0, 100000)). Host sorts each core's edges by
destination into 512-dst superblocks and 32768-row source buckets, so the
device does:
  - dma_gather of 512B feature rows (int16 bucket-relative indices)
  - segment-sum via one-hot (iota is_equal) fp32r matmuls accumulated in PSUM
  - fused W_dir @ agg + W_self @ x_own + bias*deg via fp32r matmuls
  - PE transposes to write updated row-major shards, AllGather between phases
Supervision readout is sharded by author owner (author rows local, paper rows
gathered from the AllGathered table).
"""

import numpy as np

import concourse.bacc as bacc
import concourse.mybir as mybir
import concourse.tile as tile
from concourse.bass_utils import run_bass_kernel_spmd

F32 = mybir.dt.float32
F32R = mybir.dt.float32r
F16 = mybir.dt.float16
I16 = mybir.dt.int16

NCORES = 8
D = 128
N_AUTHOR = 100000
N_PAPER_ACT = 100000          # active papers (indices ever referenced)
SHARD = N_AUTHOR // NCORES    # 12500 nodes per core (authors and papers)
SB = 512                      # superblock width (1 PSUM bank of fp32)
NSB = (SHARD + SB - 1) // SB  # 25 (24 full + one 212-wide)
MACRO = 4                     # superblocks per gather macro
NMACRO = (NSB + MACRO - 1) // MACRO
BUCKET = 32768                # int16 index range per source bucket
NBUCKET = (N_AUTHOR + BUCKET - 1) // BUCKET  # 4
WIN = 256                     # one-hot window width (non-first tiles)
PAD_DST = 5000.0              # out-of-window sentinel for pad edges
SUP_CHUNK = 8
EQ_BATCH = 8   # tiles per batched one-hot instruction
_UNUSED = 0                 # supervision gather chunk, in 128-pair tiles
N_LAYERS = 2


def _sb_width(sb):
    return min(SB, SHARD - sb * SB)


# ---------------------------------------------------------------- host prep

def _wrap_idx(idx):
    """Pack int index array (len multiple of 128) into the [128, n/16] int16
    dma_gather layout: index j at [j%16, j//16], replicated across the 8
    16-partition groups."""
    n = len(idx)
    w = np.zeros((128, n // 16), np.int16)
    base = idx.astype(np.int16).reshape(n // 16, 16).T  # [16, n/16]
    for g in range(8):
        w[16 * g:16 * g + 16, :] = base
    return w


def _build_tiles_one_core(src, dst_local):
    """Split one core's edges of one direction into gather tiles.

    Tiles are aligned to 256-wide half-windows of each superblock so the
    PSUM window base is core-independent (part of the shared program).
    Returns nt[sb][bucket][half] counts and dict (sb,b,h) -> list of
    (src_rel128, off128) unpadded tile contents."""
    sb_id = dst_local // SB
    off = dst_local - sb_id * SB
    half = off // WIN
    bucket = src // BUCKET
    rel = src - bucket * BUCKET
    tiles = {}
    nt = np.zeros((NSB, NBUCKET, 2), np.int64)
    order = np.lexsort((off, half, bucket, sb_id))
    sb_s, b_s, h_s = sb_id[order], bucket[order], half[order]
    off_s, rel_s = off[order], rel[order]
    key = (sb_s * NBUCKET + b_s) * 2 + h_s
    bounds = np.flatnonzero(np.diff(key)) + 1
    starts = np.concatenate(([0], bounds))
    ends = np.concatenate((bounds, [len(key)]))
    for s, e in zip(starts, ends):
        sb, b, h = int(sb_s[s]), int(b_s[s]), int(h_s[s])
        o = off_s[s:e]
        r = rel_s[s:e]
        group = [(r[i:i + 128], o[i:i + 128]) for i in range(0, e - s, 128)]
        tiles[(sb, b, h)] = group
        nt[sb, b, h] = len(group)
    return nt, tiles


def _emit_direction(all_tiles, global_nt):
    """Produce, for one core, the packed idx stream / dstloc array / tile meta
    given equalized per-(sb,bucket,half) tile counts global_nt.

    Returns (idx_wrapped [128, NTOT/16], dstloc [128, NTILES],
             meta list over (macro, bucket) -> list of (sb, base, width,
             first, last))."""
    idx_stream = []
    dstloc_cols = []
    meta = []          # per (m, b): list of tile tuples
    first_seen = set()
    # per-sb last tile position in (bucket, half)-major order
    last_pos = {}
    for sb in range(NSB):
        tot = int(global_nt[sb].sum())
        assert tot > 0
        c = 0
        for b in range(NBUCKET):
            for h in range(2):
                for t in range(int(global_nt[sb, b, h])):
                    c += 1
                    if c == tot:
                        last_pos[sb] = (b, h, t)
    for m in range(NMACRO):
        sbs = range(m * MACRO, min((m + 1) * MACRO, NSB))
        for b in range(NBUCKET):
            tl = []
            for sb in sbs:
                for h in range(2):
                    group = all_tiles.get((sb, b, h), [])
                    for t in range(int(global_nt[sb, b, h])):
                        if t < len(group):
                            r, o = group[t]
                        else:
                            r = np.zeros(0, np.int64)
                            o = np.zeros(0, np.int64)
                        n = len(r)
                        first = sb not in first_seen
                        if first:
                            assert h == 0
                            first_seen.add(sb)
                            base, width = 0, SB
                        else:
                            base, width = h * WIN, WIN
                        last = last_pos[sb] == (b, h, t)
                        src128 = np.zeros(128, np.int64)
                        dl128 = np.full(128, PAD_DST, np.float32)
                        src128[:n] = r
                        dl128[:n] = (o - base)[:n]
                        idx_stream.append(src128)
                        dstloc_cols.append(dl128)
                        tl.append((sb, base, width, first, last))
            meta.append(tl)
    ntiles = len(idx_stream)
    idx_flat = np.concatenate(idx_stream) if ntiles else np.zeros(0, np.int64)
    dstloc = (np.stack(dstloc_cols, axis=1) if ntiles
              else np.zeros((128, 0), np.float32))
    return _wrap_idx(idx_flat), dstloc.astype(np.float32), meta


def _prep_direction(src_all, dst_all, ncores=NCORES):
    """Full host prep of one message direction. src_all/dst_all are global
    edge arrays; dst determines owning core. Returns per-core packed arrays
    plus the (core-independent) meta."""
    owner = dst_all // SHARD
    per_core = []
    nts = []
    for k in range(ncores):
        m = owner == k
        nt, tiles = _build_tiles_one_core(src_all[m], dst_all[m] - k * SHARD)
        nts.append(nt)
        per_core.append(tiles)
    global_nt = np.maximum.reduce(nts)
    global_nt[:, 0, 0] = np.maximum(global_nt[:, 0, 0], 1)  # sb needs a first tile
    idxs, dstlocs, metas = [], [], []
    for k in range(ncores):
        iw, dl, meta = _emit_direction(per_core[k], global_nt)
        idxs.append(iw)
        dstlocs.append(dl)
        metas.append(meta)
    # degrees per destination
    degs = []
    for k in range(ncores):
        m = owner == k
        deg = np.bincount(dst_all[m] - k * SHARD, minlength=SHARD)
        degs.append(np.stack([deg, np.ones(SHARD)]).astype(np.float16))
    return idxs, dstlocs, degs, metas[0]


def _prep_sup(sup_a, sup_p, ncores=NCORES):
    """Supervision pairs sharded by author owner; sorted by paper bucket.
    Returns per-core (a_idx_wrapped, p_idx_wrapped, positions), per-bucket
    tile counts (core-uniform)."""
    owner = sup_a // SHARD
    per_core = []
    counts = np.zeros((ncores, NBUCKET), np.int64)
    for k in range(ncores):
        m = np.flatnonzero(owner == k)
        a = sup_a[m] - k * SHARD
        p = sup_p[m]
        b = p // BUCKET
        order = np.argsort(b, kind="stable")
        per_core.append((a[order], p[order], b[order], m[order]))
        for bb in range(NBUCKET):
            counts[k, bb] = int(np.ceil((b == bb).sum() / 128.0))
    gcount = np.maximum(counts.max(axis=0), 1)
    packs = []
    for k in range(ncores):
        a, p, b, pos = per_core[k]
        a_st, p_st, pos_st = [], [], []
        for bb in range(NBUCKET):
            m = b == bb
            ab, pb, posb = a[m], p[m] - bb * BUCKET, pos[m]
            n = int(gcount[bb]) * 128
            a128 = np.zeros(n, np.int64)
            p128 = np.zeros(n, np.int64)
            a128[:len(ab)] = ab
            p128[:len(pb)] = pb
            a_st.append(a128)
            p_st.append(p128)
            pos_st.append(posb)
        packs.append((_wrap_idx(np.concatenate(a_st)),
                      _wrap_idx(np.concatenate(p_st)),
                      pos_st))
    return packs, gcount


# ------------------------------------------------------------- program build

def _build_program(meta, null=False, reps=1):
    """meta: dict with keys a2p/p2a/co -> per-(macro,bucket) tile meta,
    ntiles per direction, sup gcount."""
    nc = bacc.Bacc("TRN2", target_bir_lowering=False, debug=False,
                   enable_asserts=False, num_devices=NCORES,
                   num_swdge_queues=4)
    dt_in = {}

    def din(name, shape, dt=F16):
        dt_in[name] = nc.dram_tensor(name, shape, dt, kind="ExternalInput").ap()
        return dt_in[name]

    author_t0 = din("author_t0", [N_AUTHOR, 2 * D])
    paper_t0 = din("paper_t0", [N_PAPER_ACT, 2 * D])
    xaT0 = din("xaT0", [128, SHARD])
    xpT0 = din("xpT0", [128, SHARD])
    w_cat = din("w_cat", [128, 128 * 10])
    bias_cat = din("bias_cat", [2, 128 * 6])
    iota_in = din("iota512", [128, 512], F16)
    ident_in = din("identity", [128, 128])
    dirs = ["a2p", "p2a", "co"]
    idx_in, dl_in, deg_in = {}, {}, {}
    for d in dirs:
        nt = meta[f"ntiles_{d}"]
        idx_in[d] = din(f"idx_{d}", [128, nt * 8], I16)
        dl_in[d] = din(f"dl_{d}", [128, nt], F32)
        deg_in[d] = din(f"deg_{d}", [2, SHARD])
    nsup = int(meta["sup_gcount"].sum()) * 128
    idx_sup_a = din("idx_sup_a", [128, nsup // 16], I16)
    idx_sup_p = din("idx_sup_p", [128, nsup // 16], I16)
    nsupt = nsup // 128
    out_sup = nc.dram_tensor("out_sup", [128, nsupt], F32,
                             kind="ExternalOutput").ap()

    # max gather size (tiles) over (macro, bucket) for SBUF sizing
    gmax = 1
    for d in dirs:
        for tl in meta[d]:
            gmax = max(gmax, len(tl))
    # max idx columns per macro
    idx_cols_max = 16
    for d in dirs:
        mm = meta[d]
        for m in range(NMACRO):
            c = sum(len(mm[m * NBUCKET + b]) for b in range(NBUCKET)) * 8
            idx_cols_max = max(idx_cols_max, c)

    if null:
        with tile.TileContext(nc) as tc:
            with tc.tile_pool(name="nsb", bufs=1) as sbp:
                z = sbp.tile([128, nsupt], F32, name="z")
                t0 = sbp.tile([128, 256], F16, name="t0")
                nc.sync.dma_start(out=t0[:], in_=author_t0[0:128, :])
                nc.vector.memset(z[:], 0.0)
                nc.sync.dma_start(out=out_sup[:], in_=z[:])
        nc.compile()
        return nc

    with tile.TileContext(nc) as tc:
        with tc.tile_pool(name="persist", bufs=1) as pp, \
             tc.tile_pool(name="gat", bufs=2) as gp, \
             tc.tile_pool(name="oneh", bufs=8) as sp, \
             tc.tile_pool(name="stageb", bufs=3) as bp, \
             tc.tile_pool(name="degp", bufs=4) as dgp, \
             tc.tile_pool(name="idxp", bufs=2) as ixp, \
             tc.tile_pool(name="supp", bufs=2) as sup_p, \
             tc.tile_pool(name="psA", bufs=5, space="PSUM") as psA, \
             tc.tile_pool(name="psB", bufs=2, space="PSUM") as psB, \
             tc.tile_pool(name="psT", bufs=1, space="PSUM") as psT, \
             tc.tile_pool(name="dram", bufs=1, space="DRAM") as drp:

            # ---- persistent state ----
            xaT = pp.tile([128, SHARD], F16, name="xaT")
            xpT = pp.tile([128, SHARD], F16, name="xpT")
            iota = pp.tile([128, 512], F16, name="iota")
            ident = pp.tile([128, 128], F16, name="ident")
            w_t = pp.tile([128, 128 * 10], F16, name="w_t")
            bias_t = pp.tile([2, 128 * 6], F16, name="bias_t")
            dl_t = {d: pp.tile([128, meta[f"ntiles_{d}"]], F32, name=f"dl_{d}")
                    for d in dirs}
            out_sb = pp.tile([128, nsupt], F32, name="out_sb")

            nc.sync.dma_start(out=xaT[:], in_=xaT0[:])
            nc.sync.dma_start(out=xpT[:], in_=xpT0[:])
            nc.sync.dma_start(out=iota[:], in_=iota_in[:])
            nc.sync.dma_start(out=ident[:], in_=ident_in[:])
            nc.sync.dma_start(out=w_t[:], in_=w_cat[:])
            nc.sync.dma_start(out=bias_t[:], in_=bias_cat[:])
            for d in dirs:
                nc.sync.dma_start(out=dl_t[d][:], in_=dl_in[d][:])
            idx_sup_a_t = pp.tile([128, nsup // 16], I16, name="supa")
            idx_sup_p_t = pp.tile([128, nsup // 16], I16, name="supb")
            nc.sync.dma_start(out=idx_sup_a_t[:], in_=idx_sup_a[:])
            nc.sync.dma_start(out=idx_sup_p_t[:], in_=idx_sup_p[:])

            # ---- internal DRAM tables ----
            def dram_full(name):
                return drp.tile([N_AUTHOR, 2 * D], F16, addr_space="Shared",
                                name=name)

            def dram_own(name):
                return drp.tile([SHARD, 2 * D], F16, name=name)

            a1a_own = dram_own("a1a_o")
            a1_own = dram_own("a1_o")
            p1_own = dram_own("p1_o")
            a2a_own = dram_own("a2a_o")
            p2_own = dram_own("p2_o")
            a2_own = dram_own("a2_o")

            def w_slice(l, slot):
                o = (l * 5 + slot) * 128
                return w_t[:, o:o + 128]

            def bias_slice(l, ph):
                o = (l * 3 + ph) * 128
                return bias_t[:, o:o + 128]

            def process_direction(d, l, ph, src_tbl, xown, wdir, wself,
                                  biasp, own_out, co_mode=False):
                """One direction of one layer: stage A (gather+one-hot
                matmuls), stage B per superblock, row-major writeback."""
                mm = meta[d]
                nt_dir = meta[f"ntiles_{d}"]
                deg_d = deg_in[d]
                tile_col = 0
                psum_of_sb = {}
                left_of_sb = {sb: 0 for sb in range(NSB)}
                for tl in mm:
                    for (sb, _b, _w, _f, _l) in tl:
                        left_of_sb[sb] += 1
                # idx stream column offset per macro
                col_off = 0
                for m in range(NMACRO):
                    cols = sum(len(mm[m * NBUCKET + b]) for b in range(NBUCKET)) * 8
                    if cols == 0:
                        continue
                    idx_t = ixp.tile([128, idx_cols_max], I16, tag="idx", name="idxt")
                    nc.sync.dma_start(
                        out=idx_t[:, :cols],
                        in_=idx_in[d][:, col_off:col_off + cols])
                    mac_off = 0
                    for b in range(NBUCKET):
                        tl = mm[m * NBUCKET + b]
                        ntl = len(tl)
                        if ntl == 0:
                            continue
                        nidx = ntl * 128
                        bs = b * BUCKET
                        be = min(bs + BUCKET, N_AUTHOR)
                        G = gp.tile([128, gmax * 256], F16, tag="G", name="G")
                        nc.gpsimd.dma_gather(
                            G[:, :ntl * 256].rearrange(
                                "p (c e) -> p c e", e=256),
                            src_tbl[bs:be, :],
                            idx_t[:, mac_off:mac_off + ntl * 8],
                            nidx, nidx, 256,
                            single_packet=(nidx <= 1024), queue_num=b)
                        mac_off += ntl * 8
                        for ti, (sb, base, width, first, _last) in enumerate(tl):
                            if sb not in psum_of_sb:
                                psum_of_sb[sb] = psA.tile(
                                    [128, 512], F32, tag="agg", name="agg")
                            pa = psum_of_sb[sb]
                            S = sp.tile([128, 512], F16, tag="S", name="S")
                            nc.vector.tensor_scalar(
                                out=S[:, :width], in0=iota[:, :width],
                                scalar1=dl_t[d][:, tile_col:tile_col + 1],
                                scalar2=None, op0=mybir.AluOpType.is_equal)
                            left_of_sb[sb] -= 1
                            nc.tensor.matmul(
                                out=pa[:, base:base + width],
                                lhsT=G[:, ti * 256:ti * 256 + 128],
                                rhs=S[:, :width],
                                start=first, stop=(left_of_sb[sb] == 0))
                            tile_col += 1
                    # stage B for completed superblocks of this macro
                    for sb in range(m * MACRO, min((m + 1) * MACRO, NSB)):
                        if sb not in psum_of_sb:
                            continue
                        wdt = _sb_width(sb)
                        pa = psum_of_sb.pop(sb)
                        agg_sb = bp.tile([128, 512], F16, tag="aggsb", name="aggsb")
                        nc.vector.tensor_copy(out=agg_sb[:, :wdt],
                                              in_=pa[:, :wdt])
                        deg_t = dgp.tile([2, 512], F16, tag="deg", name="degt")
                        nc.sync.dma_start(
                            out=deg_t[:, :wdt],
                            in_=deg_d[:, sb * SB:sb * SB + wdt])
                        pb = psB.tile([128, 512], F32, tag="out", name="pb")
                        nc.tensor.matmul(out=pb[:, :wdt], lhsT=wdir,
                                         rhs=agg_sb[:, :wdt],
                                         start=True, stop=False)
                        if not co_mode:
                            nc.tensor.matmul(
                                out=pb[:, :wdt], lhsT=wself,
                                rhs=xown[:, sb * SB:sb * SB + wdt],
                                start=False, stop=False)
                        nc.tensor.matmul(out=pb[:, :wdt], lhsT=biasp,
                                         rhs=deg_t[:2, :wdt],
                                         start=False, stop=True)
                        if co_mode:
                            nc.vector.tensor_tensor(
                                out=xown[:, sb * SB:sb * SB + wdt],
                                in0=pb[:, :wdt],
                                in1=xown[:, sb * SB:sb * SB + wdt],
                                op=mybir.AluOpType.add)
                        else:
                            nc.vector.tensor_copy(
                                out=xown[:, sb * SB:sb * SB + wdt],
                                in_=pb[:, :wdt])
                        # transpose to row-major and write the shard slice
                        pt = psT.tile([128, 512], F16, tag="tr", name="pt")
                        nchunk = (wdt + 127) // 128
                        for j in range(nchunk):
                            cw = min(128, wdt - j * 128)
                            nc.tensor.matmul(
                                out=pt[:cw, j * 128:j * 128 + 128],
                                lhsT=xown[:, sb * SB + j * 128:
                                          sb * SB + j * 128 + cw],
                                rhs=ident[:],
                                is_transpose=True,
                                start=(j == 0), stop=(j == nchunk - 1))
                        rm = bp.tile([128, 512], F16, tag="rm", name="rm")
                        nc.vector.tensor_copy(out=rm[:, :nchunk * 128],
                                              in_=pt[:, :nchunk * 128])
                        for j in range(nchunk):
                            cw = min(128, wdt - j * 128)
                            nc.sync.dma_start(
                                out=own_out[sb * SB + j * 128:
                                            sb * SB + j * 128 + cw, 0:128],
                                in_=rm[:cw, j * 128:j * 128 + 128])
                    col_off += cols

            def allgather(own, full):
                nc.gpsimd.collective_compute(
                    "AllGather", mybir.AluOpType.bypass,
                    replica_groups=[list(range(NCORES))],
                    ins=[own[:]], outs=[full[:]])

            # ---------------- pipeline (repeated for timing) ----------
            for _rep in range(reps):
                # Shared tiles may only have one (collective) writer; fresh
                # AG outputs per repetition
                a1a_full = dram_full(f"a1a_f{_rep}")
                a1_full = dram_full(f"a1_f{_rep}")
                p1_full = dram_full(f"p1_f{_rep}")
                a2a_full = dram_full(f"a2a_f{_rep}")
                p2_full = dram_full(f"p2_f{_rep}")
                process_direction("a2p", 0, 0, author_t0, xpT,
                                  w_slice(0, 0), w_slice(0, 1),
                                  bias_slice(0, 0), p1_own)
                process_direction("p2a", 0, 1, paper_t0, xaT,
                                  w_slice(0, 2), w_slice(0, 3),
                                  bias_slice(0, 1), a1a_own)
                allgather(a1a_own, a1a_full)
                process_direction("co", 0, 2, a1a_full[:], xaT,
                                  w_slice(0, 4), None, bias_slice(0, 2),
                                  a1_own, co_mode=True)
                allgather(a1_own, a1_full)
                allgather(p1_own, p1_full)
                process_direction("a2p", 1, 0, a1_full[:], xpT,
                                  w_slice(1, 0), w_slice(1, 1),
                                  bias_slice(1, 0), p2_own)
                process_direction("p2a", 1, 1, p1_full[:], xaT,
                                  w_slice(1, 2), w_slice(1, 3),
                                  bias_slice(1, 1), a2a_own)
                allgather(a2a_own, a2a_full)
                process_direction("co", 1, 2, a2a_full[:], xaT,
                                  w_slice(1, 4), None, bias_slice(1, 2),
                                  a2_own, co_mode=True)
                allgather(p2_own, p2_full)

                # ---------------- supervision readout ----------------
                gc = meta["sup_gcount"]
                chunks = []  # (tile_start, ntiles, bucket)
                t0 = 0
                for b in range(NBUCKET):
                    n = int(gc[b])
                    s = 0
                    while s < n:
                        c = min(SUP_CHUNK, n - s)
                        chunks.append((t0 + s, c, b))
                        s += c
                    t0 += n
                for (ts, ntl, b) in chunks:
                    nidx = ntl * 128
                    Ga = sup_p.tile([128, SUP_CHUNK * 256], F16, tag="Ga", name="Ga")
                    Gp = sup_p.tile([128, SUP_CHUNK * 256], F16, tag="Gp", name="Gp")
                    nc.gpsimd.dma_gather(
                        Ga[:, :ntl * 256].rearrange("p (c e) -> p c e", e=256),
                        a2_own[:], idx_sup_a_t[:, ts * 8:(ts + ntl) * 8],
                        nidx, nidx, 256, single_packet=(nidx <= 1024),
                        queue_num=(2 * ts) % 4)
                    bs = b * BUCKET
                    be = min(bs + BUCKET, N_AUTHOR)
                    nc.gpsimd.dma_gather(
                        Gp[:, :ntl * 256].rearrange("p (c e) -> p c e", e=256),
                        p2_full[bs:be, :], idx_sup_p_t[:, ts * 8:(ts + ntl) * 8],
                        nidx, nidx, 256, single_packet=(nidx <= 1024),
                        queue_num=(2 * ts + 1) % 4)
                    for t in range(ntl):
                        prod = sup_p.tile([128, 128], F32, tag="prod", name="prod")
                        nc.vector.tensor_tensor(
                            out=prod[:],
                            in0=Ga[:, t * 256:t * 256 + 128],
                            in1=Gp[:, t * 256:t * 256 + 128],
                            op=mybir.AluOpType.mult)
                        nc.vector.reduce_sum(
                            out=out_sb[:, ts + t:ts + t + 1], in_=prod[:],
                            axis=mybir.AxisListType.X)
            nc.sync.dma_start(out=out_sup[:], in_=out_sb[:])
    nc.compile()
    return nc


# ---------------------------------------------------------------- interface

_CACHE = {}


def _preprocess(inputs):
    xa = np.asarray(inputs["x_author"], np.float32).astype(np.float16)
    xp = np.asarray(inputs["x_paper"], np.float32)[:N_PAPER_ACT].astype(np.float16)
    xa_pad = np.zeros((N_AUTHOR, 2 * D), np.float16)
    xa_pad[:, :D] = xa
    xp_pad = np.zeros((N_PAPER_ACT, 2 * D), np.float16)
    xp_pad[:, :D] = xp
    ei = np.asarray(inputs["edge_index"], np.int64)
    ci = np.asarray(inputs["coauthor_edge_index"], np.int64)
    si = np.asarray(inputs["supervision_edge_index"], np.int64)

    idx_a2p, dl_a2p, deg_a2p, meta_a2p = _prep_direction(ei[0], ei[1])
    idx_p2a, dl_p2a, deg_p2a, meta_p2a = _prep_direction(ei[1], ei[0])
    idx_co, dl_co, deg_co, meta_co = _prep_direction(ci[0], ci[1])
    sup_packs, sup_gcount = _prep_sup(si[0], si[1])

    meta = {
        "a2p": meta_a2p, "p2a": meta_p2a, "co": meta_co,
        "ntiles_a2p": dl_a2p[0].shape[1],
        "ntiles_p2a": dl_p2a[0].shape[1],
        "ntiles_co": dl_co[0].shape[1],
        "sup_gcount": sup_gcount,
    }

    ws, bs = [], []
    for l in range(N_LAYERS):
        for nm in ["W_a2p", "W_pself", "W_p2a", "W_aself", "W_co"]:
            ws.append(np.asarray(inputs[nm], np.float32)[l].T.astype(np.float16))
        for pair in [("b_a2p", "b_pself"), ("b_p2a", "b_aself"),
                     ("b_co", None)]:
            r0 = np.asarray(inputs[pair[0]], np.float32)[l]
            r1 = (np.asarray(inputs[pair[1]], np.float32)[l]
                  if pair[1] else np.zeros(D, np.float32))
            bs.append(np.stack([r0, r1]).astype(np.float16))
    w_cat = np.concatenate(ws, axis=1)           # [128, 1280]
    # bias order: (l0: a2p, p2a, co), (l1: ...)
    bias_cat = np.concatenate(bs, axis=1)        # [2, 768]
    iota = np.broadcast_to(np.arange(512, dtype=np.float16), (128, 512)).copy()
    ident = np.eye(128, dtype=np.float16)

    in_maps = []
    for k in range(NCORES):
        in_maps.append({
            "author_t0": xa_pad,
            "paper_t0": xp_pad,
            "xaT0": xa[k * SHARD:(k + 1) * SHARD].T.copy(),
            "xpT0": xp[k * SHARD:(k + 1) * SHARD].T.copy(),
            "w_cat": w_cat, "bias_cat": bias_cat,
            "iota512": iota, "identity": ident,
            "idx_a2p": idx_a2p[k], "dl_a2p": dl_a2p[k], "deg_a2p": deg_a2p[k],
            "idx_p2a": idx_p2a[k], "dl_p2a": dl_p2a[k], "deg_p2a": deg_p2a[k],
            "idx_co": idx_co[k], "dl_co": dl_co[k], "deg_co": deg_co[k],
            "idx_sup_a": sup_packs[k][0],
            "idx_sup_p": sup_packs[k][1],
        })
    recon = [p[2] for p in sup_packs]
    return in_maps, meta, recon, si


def _postprocess(results, meta, recon):
    gc = meta["sup_gcount"]
    out = np.zeros(100000, np.float32)
    for k in range(NCORES):
        o = results[k]["out_sup"]          # [128, nsupt]
        t0 = 0
        for b in range(NBUCKET):
            pos = recon[k][b]
            n = len(pos)
            vals = o[:, t0:t0 + int(gc[b])].T.reshape(-1)[:n]
            out[pos] = vals
            t0 += int(gc[b])
    return out


def kernel(**inputs):
    in_maps, meta, recon, _si = _preprocess(inputs)
    key = "prog"
    if key not in _CACHE:
        _CACHE[key] = _build_program(meta)
    nc = _CACHE[key]
    res = run_bass_kernel_spmd(nc, in_maps, core_ids=list(range(NCORES)))
    return _postprocess(res.results, meta, recon)



# revision 7
# speedup vs baseline: 6.3427x; 6.3427x over previous
"""BipartiteGCN Trainium2 kernel (8 NeuronCores, Bass/Tile).

Strategy: shard message DESTINATIONS across cores (authors 12500/core, active
papers 12500/core; papers >= 100000 never influence the output since all paper
indices are drawn from [0, 100000)). Host sorts each core's edges by
destination into 512-dst superblocks and 32768-row source buckets, so the
device does:
  - dma_gather of 256B feature rows (int16 bucket-relative indices)
  - segment-sum via one-hot (iota is_equal) fp16 matmuls accumulated in PSUM
  - fused W_dir @ agg + W_self @ x_own + bias*deg via matmuls
  - PE transposes to write updated row-major shards, AllGather between phases
The six message phases are ordered p2a, a2p, co per layer so that every
AllGather overlaps the next (independent) message phase.
Supervision readout is sharded by author owner (author rows local, paper rows
gathered from the AllGathered table).
"""

import numpy as np

import concourse.bacc as bacc
import concourse.mybir as mybir
import concourse.tile as tile
from concourse.bass_utils import run_bass_kernel_spmd

F32 = mybir.dt.float32
F32R = mybir.dt.float32r
F16 = mybir.dt.float16
I16 = mybir.dt.int16

NCORES = 8
D = 128
N_AUTHOR = 100000
N_PAPER_ACT = 100000          # active papers (indices ever referenced)
SHARD = N_AUTHOR // NCORES    # 12500 nodes per core (authors and papers)
SB = 512                      # superblock width (1 PSUM bank of fp32)
NSB = (SHARD + SB - 1) // SB  # 25 (24 full + one 212-wide)
MACRO = 4                     # superblocks per gather macro
NMACRO = (NSB + MACRO - 1) // MACRO
BUCKET = 32768                # int16 index range per source bucket
NBUCKET = (N_AUTHOR + BUCKET - 1) // BUCKET  # 4
WIN = 256                     # one-hot window width (non-first tiles)
PAD_DST = 5000.0              # out-of-window sentinel for pad edges
SUP_CHUNK = 8
EQ_BATCH = 8   # tiles per batched one-hot instruction
_UNUSED = 0                 # supervision gather chunk, in 128-pair tiles
N_LAYERS = 2


def _sb_width(sb):
    return min(SB, SHARD - sb * SB)


# ---------------------------------------------------------------- host prep

def _wrap_idx(idx):
    """Pack int index array (len multiple of 128) into the [128, n/16] int16
    dma_gather layout: index j at [j%16, j//16], replicated across the 8
    16-partition groups."""
    n = len(idx)
    w = np.zeros((128, n // 16), np.int16)
    base = idx.astype(np.int16).reshape(n // 16, 16).T  # [16, n/16]
    for g in range(8):
        w[16 * g:16 * g + 16, :] = base
    return w


def _build_tiles_one_core(src, dst_local):
    """Split one core's edges of one direction into gather tiles.

    Tiles are aligned to 256-wide half-windows of each superblock so the
    PSUM window base is core-independent (part of the shared program).
    Returns nt[sb][bucket][half] counts and dict (sb,b,h) -> list of
    (src_rel128, off128) unpadded tile contents."""
    sb_id = dst_local // SB
    off = dst_local - sb_id * SB
    half = off // WIN
    bucket = src // BUCKET
    rel = src - bucket * BUCKET
    tiles = {}
    nt = np.zeros((NSB, NBUCKET, 2), np.int64)
    order = np.lexsort((off, half, bucket, sb_id))
    sb_s, b_s, h_s = sb_id[order], bucket[order], half[order]
    off_s, rel_s = off[order], rel[order]
    key = (sb_s * NBUCKET + b_s) * 2 + h_s
    bounds = np.flatnonzero(np.diff(key)) + 1
    starts = np.concatenate(([0], bounds))
    ends = np.concatenate((bounds, [len(key)]))
    for s, e in zip(starts, ends):
        sb, b, h = int(sb_s[s]), int(b_s[s]), int(h_s[s])
        o = off_s[s:e]
        r = rel_s[s:e]
        group = [(r[i:i + 128], o[i:i + 128]) for i in range(0, e - s, 128)]
        tiles[(sb, b, h)] = group
        nt[sb, b, h] = len(group)
    return nt, tiles


def _emit_direction(all_tiles, global_nt):
    """Produce, for one core, the packed idx stream / dstloc array / tile meta
    given equalized per-(sb,bucket,half) tile counts global_nt.

    Returns (idx_wrapped [128, NTOT/16], dstloc [128, NTILES],
             meta list over (macro, bucket) -> list of (sb, base, width,
             first, last))."""
    idx_stream = []
    dstloc_cols = []
    meta = []          # per (m, b): list of tile tuples
    first_seen = set()
    # per-sb last tile position in (bucket, half)-major order
    last_pos = {}
    for sb in range(NSB):
        tot = int(global_nt[sb].sum())
        assert tot > 0
        c = 0
        for b in range(NBUCKET):
            for h in range(2):
                for t in range(int(global_nt[sb, b, h])):
                    c += 1
                    if c == tot:
                        last_pos[sb] = (b, h, t)
    for m in range(NMACRO):
        sbs = range(m * MACRO, min((m + 1) * MACRO, NSB))
        for b in range(NBUCKET):
            tl = []
            for sb in sbs:
                for h in range(2):
                    group = all_tiles.get((sb, b, h), [])
                    for t in range(int(global_nt[sb, b, h])):
                        if t < len(group):
                            r, o = group[t]
                        else:
                            r = np.zeros(0, np.int64)
                            o = np.zeros(0, np.int64)
                        n = len(r)
                        first = sb not in first_seen
                        if first:
                            assert h == 0
                            first_seen.add(sb)
                            base, width = 0, SB
                        else:
                            base, width = h * WIN, WIN
                        last = last_pos[sb] == (b, h, t)
                        src128 = np.zeros(128, np.int64)
                        dl128 = np.full(128, PAD_DST, np.float32)
                        src128[:n] = r
                        dl128[:n] = (o - base)[:n]
                        idx_stream.append(src128)
                        dstloc_cols.append(dl128)
                        tl.append((sb, base, width, first, last))
            meta.append(tl)
    ntiles = len(idx_stream)
    idx_flat = np.concatenate(idx_stream) if ntiles else np.zeros(0, np.int64)
    dstloc = (np.stack(dstloc_cols, axis=1) if ntiles
              else np.zeros((128, 0), np.float32))
    return _wrap_idx(idx_flat), dstloc.astype(np.float32), meta


def _prep_direction(src_all, dst_all, ncores=NCORES):
    """Full host prep of one message direction. src_all/dst_all are global
    edge arrays; dst determines owning core. Returns per-core packed arrays
    plus the (core-independent) meta."""
    owner = dst_all // SHARD
    per_core = []
    nts = []
    for k in range(ncores):
        m = owner == k
        nt, tiles = _build_tiles_one_core(src_all[m], dst_all[m] - k * SHARD)
        nts.append(nt)
        per_core.append(tiles)
    global_nt = np.maximum.reduce(nts)
    global_nt[:, 0, 0] = np.maximum(global_nt[:, 0, 0], 1)  # sb needs a first tile
    idxs, dstlocs, metas = [], [], []
    for k in range(ncores):
        iw, dl, meta = _emit_direction(per_core[k], global_nt)
        idxs.append(iw)
        dstlocs.append(dl)
        metas.append(meta)
    # degrees per destination
    degs = []
    for k in range(ncores):
        m = owner == k
        deg = np.bincount(dst_all[m] - k * SHARD, minlength=SHARD)
        degs.append(np.stack([deg, np.ones(SHARD)]).astype(np.float16))
    return idxs, dstlocs, degs, metas[0]


def _prep_sup(sup_a, sup_p, ncores=NCORES):
    """Supervision pairs sharded by author owner; sorted by paper bucket.
    Returns per-core (a_idx_wrapped, p_idx_wrapped, positions), per-bucket
    tile counts (core-uniform)."""
    owner = sup_a // SHARD
    per_core = []
    counts = np.zeros((ncores, NBUCKET), np.int64)
    for k in range(ncores):
        m = np.flatnonzero(owner == k)
        a = sup_a[m] - k * SHARD
        p = sup_p[m]
        b = p // BUCKET
        order = np.argsort(b, kind="stable")
        per_core.append((a[order], p[order], b[order], m[order]))
        for bb in range(NBUCKET):
            counts[k, bb] = int(np.ceil((b == bb).sum() / 128.0))
    gcount = np.maximum(counts.max(axis=0), 1)
    packs = []
    for k in range(ncores):
        a, p, b, pos = per_core[k]
        a_st, p_st, pos_st = [], [], []
        for bb in range(NBUCKET):
            m = b == bb
            ab, pb, posb = a[m], p[m] - bb * BUCKET, pos[m]
            n = int(gcount[bb]) * 128
            a128 = np.zeros(n, np.int64)
            p128 = np.zeros(n, np.int64)
            a128[:len(ab)] = ab
            p128[:len(pb)] = pb
            a_st.append(a128)
            p_st.append(p128)
            pos_st.append(posb)
        packs.append((_wrap_idx(np.concatenate(a_st)),
                      _wrap_idx(np.concatenate(p_st)),
                      pos_st))
    return packs, gcount


# ------------------------------------------------------------- program build

def _build_program(meta, null=False, reps=1):
    """meta: dict with keys a2p/p2a/co -> per-(macro,bucket) tile meta,
    ntiles per direction, sup gcount."""
    nc = bacc.Bacc("TRN2", target_bir_lowering=False, debug=False,
                   enable_asserts=False, num_devices=NCORES,
                   num_swdge_queues=4)
    dt_in = {}

    def din(name, shape, dt=F16):
        dt_in[name] = nc.dram_tensor(name, shape, dt, kind="ExternalInput").ap()
        return dt_in[name]

    author_t0 = din("author_t0", [N_AUTHOR, D])
    paper_t0 = din("paper_t0", [N_PAPER_ACT, D])
    xaT0 = din("xaT0", [128, SHARD])
    xpT0 = din("xpT0", [128, SHARD])
    w_cat = din("w_cat", [128, 128 * 10])
    bias_cat = din("bias_cat", [2, 128 * 6])
    iota_in = din("iota512", [128, 512], F16)
    ident_in = din("identity", [128, 128])
    dirs = ["a2p", "p2a", "co"]
    idx_in, dl_in, deg_in = {}, {}, {}
    for d in dirs:
        nt = meta[f"ntiles_{d}"]
        idx_in[d] = din(f"idx_{d}", [128, nt * 8], I16)
        dl_in[d] = din(f"dl_{d}", [128, nt], F32)
        deg_in[d] = din(f"deg_{d}", [2, SHARD])
    nsup = int(meta["sup_gcount"].sum()) * 128
    idx_sup_a = din("idx_sup_a", [128, nsup // 16], I16)
    idx_sup_p = din("idx_sup_p", [128, nsup // 16], I16)
    nsupt = nsup // 128
    out_sup = nc.dram_tensor("out_sup", [128, nsupt], F32,
                             kind="ExternalOutput").ap()

    # max gather size (tiles) over (macro, bucket) for SBUF sizing
    gmax = 1
    for d in dirs:
        for tl in meta[d]:
            gmax = max(gmax, len(tl))
    # max idx columns per macro
    idx_cols_max = 16
    for d in dirs:
        mm = meta[d]
        for m in range(NMACRO):
            c = sum(len(mm[m * NBUCKET + b]) for b in range(NBUCKET)) * 8
            idx_cols_max = max(idx_cols_max, c)

    if null:
        with tile.TileContext(nc) as tc:
            with tc.tile_pool(name="nsb", bufs=1) as sbp:
                z = sbp.tile([128, nsupt], F32, name="z")
                t0 = sbp.tile([128, 128], F16, name="t0")
                nc.sync.dma_start(out=t0[:], in_=author_t0[0:128, :])
                nc.vector.memset(z[:], 0.0)
                nc.sync.dma_start(out=out_sup[:], in_=z[:])
        nc.compile()
        return nc

    with tile.TileContext(nc) as tc:
        with tc.tile_pool(name="persist", bufs=1) as pp, \
             tc.tile_pool(name="gat", bufs=2) as gp, \
             tc.tile_pool(name="oneh", bufs=8) as sp, \
             tc.tile_pool(name="stageb", bufs=3) as bp, \
             tc.tile_pool(name="degp", bufs=4) as dgp, \
             tc.tile_pool(name="idxp", bufs=2) as ixp, \
             tc.tile_pool(name="supp", bufs=2) as sup_p, \
             tc.tile_pool(name="psA", bufs=5, space="PSUM") as psA, \
             tc.tile_pool(name="psB", bufs=2, space="PSUM") as psB, \
             tc.tile_pool(name="psT", bufs=1, space="PSUM") as psT, \
             tc.tile_pool(name="dram", bufs=1, space="DRAM") as drp:

            # ---- persistent state ----
            xaT = pp.tile([128, SHARD], F16, name="xaT")
            xpT = pp.tile([128, SHARD], F16, name="xpT")
            iota = pp.tile([128, 512], F16, name="iota")
            ident = pp.tile([128, 128], F16, name="ident")
            w_t = pp.tile([128, 128 * 10], F16, name="w_t")
            bias_t = pp.tile([2, 128 * 6], F16, name="bias_t")
            dl_t = {d: pp.tile([128, meta[f"ntiles_{d}"]], F32, name=f"dl_{d}")
                    for d in dirs}
            out_sb = pp.tile([128, nsupt], F32, name="out_sb")

            nc.sync.dma_start(out=xaT[:], in_=xaT0[:])
            nc.sync.dma_start(out=xpT[:], in_=xpT0[:])
            nc.sync.dma_start(out=iota[:], in_=iota_in[:])
            nc.sync.dma_start(out=ident[:], in_=ident_in[:])
            nc.sync.dma_start(out=w_t[:], in_=w_cat[:])
            nc.sync.dma_start(out=bias_t[:], in_=bias_cat[:])
            for d in dirs:
                nc.sync.dma_start(out=dl_t[d][:], in_=dl_in[d][:])
            idx_sup_a_t = pp.tile([128, nsup // 16], I16, name="supa")
            idx_sup_p_t = pp.tile([128, nsup // 16], I16, name="supb")
            nc.sync.dma_start(out=idx_sup_a_t[:], in_=idx_sup_a[:])
            nc.sync.dma_start(out=idx_sup_p_t[:], in_=idx_sup_p[:])

            # ---- internal DRAM tables ----
            def dram_full(name):
                return drp.tile([N_AUTHOR, D], F16, addr_space="Shared",
                                name=name)

            def dram_own(name):
                return drp.tile([SHARD, D], F16, name=name)

            a1a_own = dram_own("a1a_o")
            a1_own = dram_own("a1_o")
            p1_own = dram_own("p1_o")
            a2a_own = dram_own("a2a_o")
            p2_own = dram_own("p2_o")
            a2_own = dram_own("a2_o")

            def w_slice(l, slot):
                o = (l * 5 + slot) * 128
                return w_t[:, o:o + 128]

            def bias_slice(l, ph):
                o = (l * 3 + ph) * 128
                return bias_t[:, o:o + 128]

            def process_direction(d, l, ph, src_tbl, xown, wdir, wself,
                                  biasp, own_out, co_mode=False):
                """One direction of one layer: stage A (gather+one-hot
                matmuls), stage B per superblock, row-major writeback."""
                mm = meta[d]
                nt_dir = meta[f"ntiles_{d}"]
                deg_d = deg_in[d]
                tile_col = 0
                psum_of_sb = {}
                left_of_sb = {sb: 0 for sb in range(NSB)}
                for tl in mm:
                    for (sb, _b, _w, _f, _l) in tl:
                        left_of_sb[sb] += 1
                # idx stream column offset per macro
                col_off = 0
                for m in range(NMACRO):
                    cols = sum(len(mm[m * NBUCKET + b]) for b in range(NBUCKET)) * 8
                    if cols == 0:
                        continue
                    idx_t = ixp.tile([128, idx_cols_max], I16, tag="idx", name="idxt")
                    nc.sync.dma_start(
                        out=idx_t[:, :cols],
                        in_=idx_in[d][:, col_off:col_off + cols])
                    mac_off = 0
                    for b in range(NBUCKET):
                        tl = mm[m * NBUCKET + b]
                        ntl = len(tl)
                        if ntl == 0:
                            continue
                        nidx = ntl * 128
                        bs = b * BUCKET
                        be = min(bs + BUCKET, N_AUTHOR)
                        G = gp.tile([128, gmax * 128], F16, tag="G", name="G")
                        nc.gpsimd.dma_gather(
                            G[:, :ntl * 128].rearrange(
                                "p (c e) -> p c e", e=128),
                            src_tbl[bs:be, :],
                            idx_t[:, mac_off:mac_off + ntl * 8],
                            nidx, nidx, 128,
                            single_packet=(nidx <= 1024), queue_num=b)
                        mac_off += ntl * 8
                        for ti, (sb, base, width, first, _last) in enumerate(tl):
                            if sb not in psum_of_sb:
                                psum_of_sb[sb] = psA.tile(
                                    [128, 512], F32, tag="agg", name="agg")
                            pa = psum_of_sb[sb]
                            S = sp.tile([128, 512], F16, tag="S", name="S")
                            nc.vector.tensor_scalar(
                                out=S[:, :width], in0=iota[:, :width],
                                scalar1=dl_t[d][:, tile_col:tile_col + 1],
                                scalar2=None, op0=mybir.AluOpType.is_equal)
                            left_of_sb[sb] -= 1
                            nc.tensor.matmul(
                                out=pa[:, base:base + width],
                                lhsT=G[:, ti * 128:ti * 128 + 128],
                                rhs=S[:, :width],
                                start=first, stop=(left_of_sb[sb] == 0))
                            tile_col += 1
                    # stage B for completed superblocks of this macro
                    for sb in range(m * MACRO, min((m + 1) * MACRO, NSB)):
                        if sb not in psum_of_sb:
                            continue
                        wdt = _sb_width(sb)
                        pa = psum_of_sb.pop(sb)
                        agg_sb = bp.tile([128, 512], F16, tag="aggsb", name="aggsb")
                        nc.scalar.activation(
                            out=agg_sb[:, :wdt], in_=pa[:, :wdt],
                            func=mybir.ActivationFunctionType.Copy)
                        deg_t = dgp.tile([2, 512], F16, tag="deg", name="degt")
                        nc.sync.dma_start(
                            out=deg_t[:, :wdt],
                            in_=deg_d[:, sb * SB:sb * SB + wdt])
                        pb = psB.tile([128, 512], F32, tag="out", name="pb")
                        nc.tensor.matmul(out=pb[:, :wdt], lhsT=wdir,
                                         rhs=agg_sb[:, :wdt],
                                         start=True, stop=False)
                        if not co_mode:
                            nc.tensor.matmul(
                                out=pb[:, :wdt], lhsT=wself,
                                rhs=xown[:, sb * SB:sb * SB + wdt],
                                start=False, stop=False)
                        nc.tensor.matmul(out=pb[:, :wdt], lhsT=biasp,
                                         rhs=deg_t[:2, :wdt],
                                         start=False, stop=True)
                        if co_mode:
                            nc.vector.tensor_tensor(
                                out=xown[:, sb * SB:sb * SB + wdt],
                                in0=pb[:, :wdt],
                                in1=xown[:, sb * SB:sb * SB + wdt],
                                op=mybir.AluOpType.add)
                        else:
                            nc.scalar.activation(
                                out=xown[:, sb * SB:sb * SB + wdt],
                                in_=pb[:, :wdt],
                                func=mybir.ActivationFunctionType.Copy)
                        # transpose to row-major and write the shard slice
                        pt = psT.tile([128, 512], F16, tag="tr", name="pt")
                        nchunk = (wdt + 127) // 128
                        for j in range(nchunk):
                            cw = min(128, wdt - j * 128)
                            nc.tensor.matmul(
                                out=pt[:cw, j * 128:j * 128 + 128],
                                lhsT=xown[:, sb * SB + j * 128:
                                          sb * SB + j * 128 + cw],
                                rhs=ident[:],
                                is_transpose=True,
                                start=(j == 0), stop=(j == nchunk - 1))
                        rm = bp.tile([128, 512], F16, tag="rm", name="rm")
                        nc.scalar.activation(
                            out=rm[:, :nchunk * 128], in_=pt[:, :nchunk * 128],
                            func=mybir.ActivationFunctionType.Copy)
                        for j in range(nchunk):
                            cw = min(128, wdt - j * 128)
                            nc.sync.dma_start(
                                out=own_out[sb * SB + j * 128:
                                            sb * SB + j * 128 + cw, 0:128],
                                in_=rm[:cw, j * 128:j * 128 + 128])
                    col_off += cols

            def allgather(own, full):
                nc.gpsimd.collective_compute(
                    "AllGather", mybir.AluOpType.bypass,
                    replica_groups=[list(range(NCORES))],
                    ins=[own[:]], outs=[full[:]])

            # ---------------- pipeline (repeated for timing) ----------
            for _rep in range(reps):
                # Shared tiles may only have one (collective) writer; fresh
                # AG outputs per repetition
                a1a_full = dram_full(f"a1a_f{_rep}")
                a1_full = dram_full(f"a1_f{_rep}")
                p1_full = dram_full(f"p1_f{_rep}")
                a2a_full = dram_full(f"a2a_f{_rep}")
                p2_full = dram_full(f"p2_f{_rep}")
                # Order pairs every AllGather with an independent message
                # phase so the collective transfer hides under compute:
                #   p2a(0); AG(a1a) || a2p(0); AG(p1) || co(0);
                #   AG(a1) || p2a(1); AG(a2a) || a2p(1); AG(p2) || co(1); sup
                process_direction("p2a", 0, 1, paper_t0, xaT,
                                  w_slice(0, 2), w_slice(0, 3),
                                  bias_slice(0, 1), a1a_own)
                allgather(a1a_own, a1a_full)
                process_direction("a2p", 0, 0, author_t0, xpT,
                                  w_slice(0, 0), w_slice(0, 1),
                                  bias_slice(0, 0), p1_own)
                allgather(p1_own, p1_full)
                process_direction("co", 0, 2, a1a_full[:], xaT,
                                  w_slice(0, 4), None, bias_slice(0, 2),
                                  a1_own, co_mode=True)
                allgather(a1_own, a1_full)
                process_direction("p2a", 1, 1, p1_full[:], xaT,
                                  w_slice(1, 2), w_slice(1, 3),
                                  bias_slice(1, 1), a2a_own)
                allgather(a2a_own, a2a_full)
                process_direction("a2p", 1, 0, a1_full[:], xpT,
                                  w_slice(1, 0), w_slice(1, 1),
                                  bias_slice(1, 0), p2_own)
                allgather(p2_own, p2_full)
                process_direction("co", 1, 2, a2a_full[:], xaT,
                                  w_slice(1, 4), None, bias_slice(1, 2),
                                  a2_own, co_mode=True)

                # ---------------- supervision readout ----------------
                gc = meta["sup_gcount"]
                chunks = []  # (tile_start, ntiles, bucket)
                t0 = 0
                for b in range(NBUCKET):
                    n = int(gc[b])
                    s = 0
                    while s < n:
                        c = min(SUP_CHUNK, n - s)
                        chunks.append((t0 + s, c, b))
                        s += c
                    t0 += n
                for (ts, ntl, b) in chunks:
                    nidx = ntl * 128
                    Ga = sup_p.tile([128, SUP_CHUNK * 128], F16, tag="Ga", name="Ga")
                    Gp = sup_p.tile([128, SUP_CHUNK * 128], F16, tag="Gp", name="Gp")
                    nc.gpsimd.dma_gather(
                        Ga[:, :ntl * 128].rearrange("p (c e) -> p c e", e=128),
                        a2_own[:], idx_sup_a_t[:, ts * 8:(ts + ntl) * 8],
                        nidx, nidx, 128, single_packet=(nidx <= 1024),
                        queue_num=(2 * ts) % 4)
                    bs = b * BUCKET
                    be = min(bs + BUCKET, N_AUTHOR)
                    nc.gpsimd.dma_gather(
                        Gp[:, :ntl * 128].rearrange("p (c e) -> p c e", e=128),
                        p2_full[bs:be, :], idx_sup_p_t[:, ts * 8:(ts + ntl) * 8],
                        nidx, nidx, 128, single_packet=(nidx <= 1024),
                        queue_num=(2 * ts + 1) % 4)
                    for t in range(ntl):
                        prod = sup_p.tile([128, 128], F32, tag="prod", name="prod")
                        nc.vector.tensor_tensor(
                            out=prod[:],
                            in0=Ga[:, t * 128:t * 128 + 128],
                            in1=Gp[:, t * 128:t * 128 + 128],
                            op=mybir.AluOpType.mult)
                        nc.vector.reduce_sum(
                            out=out_sb[:, ts + t:ts + t + 1], in_=prod[:],
                            axis=mybir.AxisListType.X)
            nc.sync.dma_start(out=out_sup[:], in_=out_sb[:])
    nc.compile()
    return nc


# ---------------------------------------------------------------- interface

_CACHE = {}


def _preprocess(inputs):
    xa = np.asarray(inputs["x_author"], np.float32).astype(np.float16)
    xp = np.asarray(inputs["x_paper"], np.float32)[:N_PAPER_ACT].astype(np.float16)
    ei = np.asarray(inputs["edge_index"], np.int64)
    ci = np.asarray(inputs["coauthor_edge_index"], np.int64)
    si = np.asarray(inputs["supervision_edge_index"], np.int64)

    idx_a2p, dl_a2p, deg_a2p, meta_a2p = _prep_direction(ei[0], ei[1])
    idx_p2a, dl_p2a, deg_p2a, meta_p2a = _prep_direction(ei[1], ei[0])
    idx_co, dl_co, deg_co, meta_co = _prep_direction(ci[0], ci[1])
    sup_packs, sup_gcount = _prep_sup(si[0], si[1])

    meta = {
        "a2p": meta_a2p, "p2a": meta_p2a, "co": meta_co,
        "ntiles_a2p": dl_a2p[0].shape[1],
        "ntiles_p2a": dl_p2a[0].shape[1],
        "ntiles_co": dl_co[0].shape[1],
        "sup_gcount": sup_gcount,
    }

    ws, bs = [], []
    for l in range(N_LAYERS):
        for nm in ["W_a2p", "W_pself", "W_p2a", "W_aself", "W_co"]:
            ws.append(np.asarray(inputs[nm], np.float32)[l].T.astype(np.float16))
        for pair in [("b_a2p", "b_pself"), ("b_p2a", "b_aself"),
                     ("b_co", None)]:
            r0 = np.asarray(inputs[pair[0]], np.float32)[l]
            r1 = (np.asarray(inputs[pair[1]], np.float32)[l]
                  if pair[1] else np.zeros(D, np.float32))
            bs.append(np.stack([r0, r1]).astype(np.float16))
    w_cat = np.concatenate(ws, axis=1)           # [128, 1280]
    # bias order: (l0: a2p, p2a, co), (l1: ...)
    bias_cat = np.concatenate(bs, axis=1)        # [2, 768]
    iota = np.broadcast_to(np.arange(512, dtype=np.float16), (128, 512)).copy()
    ident = np.eye(128, dtype=np.float16)

    in_maps = []
    for k in range(NCORES):
        in_maps.append({
            "author_t0": np.ascontiguousarray(xa),
            "paper_t0": np.ascontiguousarray(xp),
            "xaT0": xa[k * SHARD:(k + 1) * SHARD].T.copy(),
            "xpT0": xp[k * SHARD:(k + 1) * SHARD].T.copy(),
            "w_cat": w_cat, "bias_cat": bias_cat,
            "iota512": iota, "identity": ident,
            "idx_a2p": idx_a2p[k], "dl_a2p": dl_a2p[k], "deg_a2p": deg_a2p[k],
            "idx_p2a": idx_p2a[k], "dl_p2a": dl_p2a[k], "deg_p2a": deg_p2a[k],
            "idx_co": idx_co[k], "dl_co": dl_co[k], "deg_co": deg_co[k],
            "idx_sup_a": sup_packs[k][0],
            "idx_sup_p": sup_packs[k][1],
        })
    recon = [p[2] for p in sup_packs]
    return in_maps, meta, recon, si


def _postprocess(results, meta, recon):
    gc = meta["sup_gcount"]
    out = np.zeros(100000, np.float32)
    for k in range(NCORES):
        o = results[k]["out_sup"]          # [128, nsupt]
        t0 = 0
        for b in range(NBUCKET):
            pos = recon[k][b]
            n = len(pos)
            vals = o[:, t0:t0 + int(gc[b])].T.reshape(-1)[:n]
            out[pos] = vals
            t0 += int(gc[b])
    return out


def kernel(**inputs):
    in_maps, meta, recon, _si = _preprocess(inputs)
    key = "prog"
    if key not in _CACHE:
        _CACHE[key] = _build_program(meta)
    nc = _CACHE[key]
    res = run_bass_kernel_spmd(nc, in_maps, core_ids=list(range(NCORES)))
    return _postprocess(res.results, meta, recon)



# revision 15
# speedup vs baseline: 8.3897x; 1.3227x over previous
"""BipartiteGCN Trainium2 kernel (8 NeuronCores, Bass/Tile).

Strategy: shard message DESTINATIONS across cores (authors 12500/core, active
papers 12500/core; papers >= 100000 never influence the output since all paper
indices are drawn from [0, 100000)). Host sorts each core's edges by
destination into 512-dst superblocks and 32768-row source buckets, so the
device does:
  - dma_gather of 256B feature rows (int16 bucket-relative indices)
  - segment-sum via one-hot (iota is_equal) fp16 matmuls accumulated in PSUM
  - fused W_dir @ agg + W_self @ x_own + bias*deg via matmuls
  - PE transposes to write updated row-major shards, AllGather between phases
The six message phases are ordered p2a, a2p, co per layer so that every
AllGather overlaps the next (independent) message phase.
Supervision readout is sharded by author owner (author rows local, paper rows
gathered from the AllGathered table).
"""

import numpy as np

import concourse.bacc as bacc
import concourse.mybir as mybir
import concourse.tile as tile
from concourse.bass import AP
from concourse.bass_utils import run_bass_kernel_spmd

F32 = mybir.dt.float32
F32R = mybir.dt.float32r
F16 = mybir.dt.float16
I16 = mybir.dt.int16

NCORES = 8
D = 128
N_AUTHOR = 100000
N_PAPER_ACT = 100000          # active papers (indices ever referenced)
SHARD = N_AUTHOR // NCORES    # 12500 nodes per core (authors and papers)
SB = 512                      # superblock width (1 PSUM bank of fp32)
NSB = (SHARD + SB - 1) // SB  # 25 (24 full + one 212-wide)
MACRO = 4                     # superblocks per gather macro
NMACRO = (NSB + MACRO - 1) // MACRO
BUCKET = 32768                # int16 index range per source bucket
NBUCKET = (N_AUTHOR + BUCKET - 1) // BUCKET  # 4
WIN = 256                     # one-hot window width (non-first tiles)
PAD_DST = 5000.0              # out-of-window sentinel for pad edges
SUP_CHUNK = 8
EQ_BATCH = 4   # tiles per batched one-hot instruction
_UNUSED = 0                 # supervision gather chunk, in 128-pair tiles
N_LAYERS = 2


def _sb_width(sb):
    return min(SB, SHARD - sb * SB)


# ---------------------------------------------------------------- host prep

def _wrap_idx(idx):
    """Pack int index array (len multiple of 128) into the [128, n/16] int16
    dma_gather layout: index j at [j%16, j//16], replicated across the 8
    16-partition groups."""
    n = len(idx)
    w = np.zeros((128, n // 16), np.int16)
    base = idx.astype(np.int16).reshape(n // 16, 16).T  # [16, n/16]
    for g in range(8):
        w[16 * g:16 * g + 16, :] = base
    return w


def _build_tiles_one_core(src, dst_local):
    """Split one core's edges of one direction into gather tiles.

    Tiles are grouped by (superblock, source bucket); every tile targets the
    full 512-wide PSUM window of its superblock. Returns nt[sb][bucket]
    counts and dict (sb,b) -> list of (src_rel128, off128) tile contents."""
    sb_id = dst_local // SB
    off = dst_local - sb_id * SB
    bucket = src // BUCKET
    rel = src - bucket * BUCKET
    tiles = {}
    nt = np.zeros((NSB, NBUCKET), np.int64)
    order = np.lexsort((off, bucket, sb_id))
    sb_s, b_s = sb_id[order], bucket[order]
    off_s, rel_s = off[order], rel[order]
    key = sb_s * NBUCKET + b_s
    bounds = np.flatnonzero(np.diff(key)) + 1
    starts = np.concatenate(([0], bounds))
    ends = np.concatenate((bounds, [len(key)]))
    for s, e in zip(starts, ends):
        sb, b = int(sb_s[s]), int(b_s[s])
        o = off_s[s:e]
        r = rel_s[s:e]
        group = [(r[i:i + 128], o[i:i + 128]) for i in range(0, e - s, 128)]
        tiles[(sb, b)] = group
        nt[sb, b] = len(group)
    return nt, tiles


def _emit_direction(all_tiles, global_nt):
    """Produce, for one core, the packed idx stream / dstloc array / tile meta
    given equalized per-(sb,bucket) tile counts global_nt.

    Returns (idx_wrapped [128, NTOT/16], dstloc [128, NTILES],
             meta list over (macro, bucket) -> list of (sb, first, last))."""
    idx_stream = []
    dstloc_cols = []
    meta = []          # per (m, b): list of tile tuples
    first_seen = set()
    # per-sb last tile position in bucket-major order
    last_pos = {}
    for sb in range(NSB):
        tot = int(global_nt[sb].sum())
        assert tot > 0
        c = 0
        for b in range(NBUCKET):
            for t in range(int(global_nt[sb, b])):
                c += 1
                if c == tot:
                    last_pos[sb] = (b, t)
    for m in range(NMACRO):
        sbs = range(m * MACRO, min((m + 1) * MACRO, NSB))
        for b in range(NBUCKET):
            tl = []
            for sb in sbs:
                group = all_tiles.get((sb, b), [])
                for t in range(int(global_nt[sb, b])):
                    if t < len(group):
                        r, o = group[t]
                    else:
                        r = np.zeros(0, np.int64)
                        o = np.zeros(0, np.int64)
                    n = len(r)
                    first = sb not in first_seen
                    if first:
                        first_seen.add(sb)
                    last = last_pos[sb] == (b, t)
                    src128 = np.zeros(128, np.int64)
                    dl128 = np.full(128, PAD_DST, np.float16)
                    src128[:n] = r
                    dl128[:n] = o[:n]
                    idx_stream.append(src128)
                    dstloc_cols.append(dl128)
                    tl.append((sb, first, last))
            meta.append(tl)
    ntiles = len(idx_stream)
    idx_flat = np.concatenate(idx_stream) if ntiles else np.zeros(0, np.int64)
    dstloc = (np.stack(dstloc_cols, axis=1) if ntiles
              else np.zeros((128, 0), np.float16))
    return _wrap_idx(idx_flat), dstloc.astype(np.float16), meta


def _prep_direction(src_all, dst_all, ncores=NCORES):
    """Full host prep of one message direction. src_all/dst_all are global
    edge arrays; dst determines owning core. Returns per-core packed arrays
    plus the (core-independent) meta."""
    owner = dst_all // SHARD
    per_core = []
    nts = []
    for k in range(ncores):
        m = owner == k
        nt, tiles = _build_tiles_one_core(src_all[m], dst_all[m] - k * SHARD)
        nts.append(nt)
        per_core.append(tiles)
    global_nt = np.maximum.reduce(nts)
    global_nt[:, 0] = np.maximum(global_nt[:, 0], 1)  # sb needs a first tile
    idxs, dstlocs, metas = [], [], []
    for k in range(ncores):
        iw, dl, meta = _emit_direction(per_core[k], global_nt)
        idxs.append(iw)
        dstlocs.append(dl)
        metas.append(meta)
    # degrees per destination
    degs = []
    for k in range(ncores):
        m = owner == k
        deg = np.bincount(dst_all[m] - k * SHARD, minlength=SHARD)
        degs.append(np.stack([deg, np.ones(SHARD)]).astype(np.float16))
    return idxs, dstlocs, degs, metas[0]


def _prep_sup(sup_a, sup_p, ncores=NCORES):
    """Supervision pairs sharded by author owner; sorted by paper bucket.
    Returns per-core (a_idx_wrapped, p_idx_wrapped, positions), per-bucket
    tile counts (core-uniform)."""
    owner = sup_a // SHARD
    per_core = []
    counts = np.zeros((ncores, NBUCKET), np.int64)
    for k in range(ncores):
        m = np.flatnonzero(owner == k)
        a = sup_a[m] - k * SHARD
        p = sup_p[m]
        b = p // BUCKET
        order = np.argsort(b, kind="stable")
        per_core.append((a[order], p[order], b[order], m[order]))
        for bb in range(NBUCKET):
            counts[k, bb] = int(np.ceil((b == bb).sum() / 128.0))
    gcount = np.maximum(counts.max(axis=0), 1)
    packs = []
    for k in range(ncores):
        a, p, b, pos = per_core[k]
        a_st, p_st, pos_st = [], [], []
        for bb in range(NBUCKET):
            m = b == bb
            ab, pb, posb = a[m], p[m] - bb * BUCKET, pos[m]
            n = int(gcount[bb]) * 128
            a128 = np.zeros(n, np.int64)
            p128 = np.zeros(n, np.int64)
            a128[:len(ab)] = ab
            p128[:len(pb)] = pb
            a_st.append(a128)
            p_st.append(p128)
            pos_st.append(posb)
        packs.append((_wrap_idx(np.concatenate(a_st)),
                      _wrap_idx(np.concatenate(p_st)),
                      pos_st))
    return packs, gcount


# ------------------------------------------------------------- program build

def _build_program(meta, null=False, reps=1, ablate=None):
    """meta: dict with keys a2p/p2a/co -> per-(macro,bucket) tile meta,
    ntiles per direction, sup gcount."""
    nc = bacc.Bacc("TRN2", target_bir_lowering=False, debug=False,
                   enable_asserts=False, num_devices=NCORES,
                   num_swdge_queues=4)
    dt_in = {}

    def din(name, shape, dt=F16):
        dt_in[name] = nc.dram_tensor(name, shape, dt, kind="ExternalInput").ap()
        return dt_in[name]

    author_t0 = din("author_t0", [N_AUTHOR, D])
    paper_t0 = din("paper_t0", [N_PAPER_ACT, D])
    xaT0 = din("xaT0", [128, SHARD])
    xpT0 = din("xpT0", [128, SHARD])
    w_cat = din("w_cat", [128, 128 * 10])
    bias_cat = din("bias_cat", [2, 128 * 6])
    iota_in = din("iota512", [128, 512], F16)
    ident_in = din("identity", [128, 128])
    dirs = ["a2p", "p2a", "co"]
    idx_in, dl_in, deg_in = {}, {}, {}
    for d in dirs:
        nt = meta[f"ntiles_{d}"]
        idx_in[d] = din(f"idx_{d}", [128, nt * 8], I16)
        dl_in[d] = din(f"dl_{d}", [128, nt], F16)
        deg_in[d] = din(f"deg_{d}", [2, SHARD])
    nsup = int(meta["sup_gcount"].sum()) * 128
    idx_sup_a = din("idx_sup_a", [128, nsup // 16], I16)
    idx_sup_p = din("idx_sup_p", [128, nsup // 16], I16)
    nsupt = nsup // 128
    out_sup = nc.dram_tensor("out_sup", [128, nsupt], F32,
                             kind="ExternalOutput").ap()

    # max gather size (tiles) over (macro, bucket) for SBUF sizing
    gmax = 1
    for d in dirs:
        for tl in meta[d]:
            gmax = max(gmax, len(tl))
    # max idx columns per macro
    idx_cols_max = 16
    for d in dirs:
        mm = meta[d]
        for m in range(NMACRO):
            c = sum(len(mm[m * NBUCKET + b]) for b in range(NBUCKET)) * 8
            idx_cols_max = max(idx_cols_max, c)

    if null:
        with tile.TileContext(nc) as tc:
            with tc.tile_pool(name="nsb", bufs=1) as sbp:
                z = sbp.tile([128, nsupt], F32, name="z")
                t0 = sbp.tile([128, 128], F16, name="t0")
                nc.sync.dma_start(out=t0[:], in_=author_t0[0:128, :])
                nc.vector.memset(z[:], 0.0)
                nc.sync.dma_start(out=out_sup[:], in_=z[:])
        nc.compile()
        return nc

    with tile.TileContext(nc) as tc:
        with tc.tile_pool(name="persist", bufs=1) as pp, \
             tc.tile_pool(name="gat", bufs=2) as gp, \
             tc.tile_pool(name="oneh", bufs=6) as sp, \
             tc.tile_pool(name="stageb", bufs=3) as bp, \
             tc.tile_pool(name="degp", bufs=4) as dgp, \
             tc.tile_pool(name="idxp", bufs=2) as ixp, \
             tc.tile_pool(name="supp", bufs=2) as sup_p, \
             tc.tile_pool(name="psA", bufs=5, space="PSUM") as psA, \
             tc.tile_pool(name="psB", bufs=2, space="PSUM") as psB, \
             tc.tile_pool(name="psT", bufs=1, space="PSUM") as psT, \
             tc.tile_pool(name="dram", bufs=1, space="DRAM") as drp:

            # ---- persistent state ----
            xaT = pp.tile([128, SHARD], F16, name="xaT")
            xpT = pp.tile([128, SHARD], F16, name="xpT")
            iota = pp.tile([128, 512], F16, name="iota")
            ident = pp.tile([128, 128], F16, name="ident")
            w_t = pp.tile([128, 128 * 10], F16, name="w_t")
            bias_t = pp.tile([2, 128 * 6], F16, name="bias_t")
            dl_t = {d: pp.tile([128, meta[f"ntiles_{d}"]], F16, name=f"dl_{d}")
                    for d in dirs}
            out_sb = pp.tile([128, nsupt], F32, name="out_sb")
            if ablate == "agonly":
                nc.vector.memset(out_sb[:], 0.0)

            nc.sync.dma_start(out=xaT[:], in_=xaT0[:])
            nc.sync.dma_start(out=xpT[:], in_=xpT0[:])
            nc.sync.dma_start(out=iota[:], in_=iota_in[:])
            nc.sync.dma_start(out=ident[:], in_=ident_in[:])
            nc.sync.dma_start(out=w_t[:], in_=w_cat[:])
            nc.sync.dma_start(out=bias_t[:], in_=bias_cat[:])
            for d in dirs:
                nc.sync.dma_start(out=dl_t[d][:], in_=dl_in[d][:])
            idx_sup_a_t = pp.tile([128, nsup // 16], I16, name="supa")
            idx_sup_p_t = pp.tile([128, nsup // 16], I16, name="supb")
            nc.sync.dma_start(out=idx_sup_a_t[:], in_=idx_sup_a[:])
            nc.sync.dma_start(out=idx_sup_p_t[:], in_=idx_sup_p[:])

            # ---- internal DRAM tables ----
            def dram_full(name):
                return drp.tile([N_AUTHOR, D], F16, addr_space="Shared",
                                name=name)

            def dram_own(name):
                return drp.tile([SHARD, D], F16, name=name)

            a1a_own = dram_own("a1a_o")
            a1_own = dram_own("a1_o")
            p1_own = dram_own("p1_o")
            a2a_own = dram_own("a2a_o")
            p2_own = dram_own("p2_o")
            a2_own = dram_own("a2_o")

            def w_slice(l, slot):
                o = (l * 5 + slot) * 128
                return w_t[:, o:o + 128]

            def bias_slice(l, ph):
                o = (l * 3 + ph) * 128
                return bias_t[:, o:o + 128]

            def process_direction(d, l, ph, src_tbl, xown, wdir, wself,
                                  biasp, own_out, co_mode=False):
                """One direction of one layer: stage A (gather+one-hot
                matmuls), stage B per superblock, row-major writeback."""
                if ablate == "agonly":
                    return
                mm = meta[d]
                nt_dir = meta[f"ntiles_{d}"]
                deg_d = deg_in[d]
                tile_col = 0
                psum_of_sb = {}
                left_of_sb = {sb: 0 for sb in range(NSB)}
                for tl in mm:
                    for (sb, _f, _l) in tl:
                        left_of_sb[sb] += 1
                # idx stream column offset per macro
                col_off = 0
                for m in range(NMACRO):
                    cols = sum(len(mm[m * NBUCKET + b]) for b in range(NBUCKET)) * 8
                    if cols == 0:
                        continue
                    idx_t = ixp.tile([128, idx_cols_max], I16, tag="idx", name="idxt")
                    nc.sync.dma_start(
                        out=idx_t[:, :cols],
                        in_=idx_in[d][:, col_off:col_off + cols])
                    mac_off = 0
                    for b in range(NBUCKET):
                        tl = mm[m * NBUCKET + b]
                        ntl = len(tl)
                        if ntl == 0:
                            continue
                        nidx = ntl * 128
                        bs = b * BUCKET
                        be = min(bs + BUCKET, N_AUTHOR)
                        G = gp.tile([128, gmax * 128], F16, tag="G", name="G")
                        nc.gpsimd.dma_gather(
                            G[:, :ntl * 128].rearrange(
                                "p (c e) -> p c e", e=128),
                            src_tbl[bs:be, :],
                            idx_t[:, mac_off:mac_off + ntl * 8],
                            nidx, nidx, 128,
                            single_packet=(nidx <= 1024), queue_num=b)
                        mac_off += ntl * 8
                        # batched one-hot: S[p, t*512+j] =
                        #   (iota[p, j] == dl[p, tile_col+t]) for runs of
                        #   EQ_BATCH tiles in one stride-0 tensor_tensor
                        S_of = {}
                        for r0 in range(0, ntl, EQ_BATCH):
                            n = min(EQ_BATCH, ntl - r0)
                            S = sp.tile([128, EQ_BATCH * 512], F16,
                                        tag="S", name="S")
                            iap = iota[:, :512]
                            dap = dl_t[d][:, tile_col + r0:tile_col + r0 + n]
                            oap = S[:, :n * 512]
                            nc.vector.tensor_tensor(
                                out=AP(oap.tensor, oap.offset,
                                       [oap.ap[0], [512, n], [1, 512]]),
                                in0=AP(iap.tensor, iap.offset,
                                       [iap.ap[0], [0, n], iap.ap[1]]),
                                in1=AP(dap.tensor, dap.offset,
                                       [dap.ap[0], dap.ap[1], [0, 512]]),
                                op=mybir.AluOpType.is_equal)
                            for t in range(n):
                                S_of[r0 + t] = (S, t * 512)
                        for ti, (sb, first, _last) in enumerate(tl):
                            if sb not in psum_of_sb:
                                psum_of_sb[sb] = psA.tile(
                                    [128, 512], F32, tag="agg", name="agg")
                            pa = psum_of_sb[sb]
                            S, so = S_of[ti]
                            left_of_sb[sb] -= 1
                            nc.tensor.matmul(
                                out=pa[:, :],
                                lhsT=G[:, ti * 128:ti * 128 + 128],
                                rhs=S[:, so:so + 512],
                                start=first, stop=(left_of_sb[sb] == 0))
                            tile_col += 1
                    # stage B for completed superblocks of this macro
                    for sb in range(m * MACRO, min((m + 1) * MACRO, NSB)):
                        if sb not in psum_of_sb:
                            continue
                        wdt = _sb_width(sb)
                        pa = psum_of_sb.pop(sb)
                        agg_sb = bp.tile([128, 512], F16, tag="aggsb", name="aggsb")
                        nc.scalar.activation(
                            out=agg_sb[:, :wdt], in_=pa[:, :wdt],
                            func=mybir.ActivationFunctionType.Copy)
                        deg_t = dgp.tile([2, 512], F16, tag="deg", name="degt")
                        nc.sync.dma_start(
                            out=deg_t[:, :wdt],
                            in_=deg_d[:, sb * SB:sb * SB + wdt])
                        pb = psB.tile([128, 512], F32, tag="out", name="pb")
                        nc.tensor.matmul(out=pb[:, :wdt], lhsT=wdir,
                                         rhs=agg_sb[:, :wdt],
                                         start=True, stop=False)
                        if not co_mode:
                            nc.tensor.matmul(
                                out=pb[:, :wdt], lhsT=wself,
                                rhs=xown[:, sb * SB:sb * SB + wdt],
                                start=False, stop=False)
                        nc.tensor.matmul(out=pb[:, :wdt], lhsT=biasp,
                                         rhs=deg_t[:2, :wdt],
                                         start=False, stop=True)
                        if co_mode:
                            nc.vector.tensor_tensor(
                                out=xown[:, sb * SB:sb * SB + wdt],
                                in0=pb[:, :wdt],
                                in1=xown[:, sb * SB:sb * SB + wdt],
                                op=mybir.AluOpType.add)
                        else:
                            nc.scalar.activation(
                                out=xown[:, sb * SB:sb * SB + wdt],
                                in_=pb[:, :wdt],
                                func=mybir.ActivationFunctionType.Copy)
                        # transpose to row-major and write the shard slice
                        pt = psT.tile([128, 512], F16, tag="tr", name="pt")
                        nchunk = (wdt + 127) // 128
                        for j in range(nchunk):
                            cw = min(128, wdt - j * 128)
                            nc.tensor.matmul(
                                out=pt[:cw, j * 128:j * 128 + 128],
                                lhsT=xown[:, sb * SB + j * 128:
                                          sb * SB + j * 128 + cw],
                                rhs=ident[:],
                                is_transpose=True,
                                start=(j == 0), stop=(j == nchunk - 1))
                        rm = bp.tile([128, 512], F16, tag="rm", name="rm")
                        nc.scalar.activation(
                            out=rm[:, :nchunk * 128], in_=pt[:, :nchunk * 128],
                            func=mybir.ActivationFunctionType.Copy)
                        for j in range(nchunk):
                            cw = min(128, wdt - j * 128)
                            nc.sync.dma_start(
                                out=own_out[sb * SB + j * 128:
                                            sb * SB + j * 128 + cw, 0:128],
                                in_=rm[:cw, j * 128:j * 128 + 128])
                    col_off += cols

            def allgather(own, full):
                if ablate == "noag":
                    return
                nc.gpsimd.collective_compute(
                    "AllGather", mybir.AluOpType.bypass,
                    replica_groups=[list(range(NCORES))],
                    ins=[own[:]], outs=[full[:]])

            # ---------------- pipeline (repeated for timing) ----------
            for _rep in range(reps):
                # Shared tiles may only have one (collective) writer; fresh
                # AG outputs per repetition
                a1a_full = dram_full(f"a1a_f{_rep}")
                a1_full = dram_full(f"a1_f{_rep}")
                p1_full = dram_full(f"p1_f{_rep}")
                a2a_full = dram_full(f"a2a_f{_rep}")
                p2_full = dram_full(f"p2_f{_rep}")
                # Order pairs every AllGather with an independent message
                # phase so the collective transfer hides under compute:
                #   p2a(0); AG(a1a) || a2p(0); AG(p1) || co(0);
                #   AG(a1) || p2a(1); AG(a2a) || a2p(1); AG(p2) || co(1); sup
                process_direction("p2a", 0, 1, paper_t0, xaT,
                                  w_slice(0, 2), w_slice(0, 3),
                                  bias_slice(0, 1), a1a_own)
                allgather(a1a_own, a1a_full)
                process_direction("a2p", 0, 0, author_t0, xpT,
                                  w_slice(0, 0), w_slice(0, 1),
                                  bias_slice(0, 0), p1_own)
                allgather(p1_own, p1_full)
                process_direction("co", 0, 2, a1a_full[:], xaT,
                                  w_slice(0, 4), None, bias_slice(0, 2),
                                  a1_own, co_mode=True)
                allgather(a1_own, a1_full)
                process_direction("p2a", 1, 1, p1_full[:], xaT,
                                  w_slice(1, 2), w_slice(1, 3),
                                  bias_slice(1, 1), a2a_own)
                allgather(a2a_own, a2a_full)
                process_direction("a2p", 1, 0, a1_full[:], xpT,
                                  w_slice(1, 0), w_slice(1, 1),
                                  bias_slice(1, 0), p2_own)
                allgather(p2_own, p2_full)
                process_direction("co", 1, 2, a2a_full[:], xaT,
                                  w_slice(1, 4), None, bias_slice(1, 2),
                                  a2_own, co_mode=True)

                # ---------------- supervision readout ----------------
                gc = meta["sup_gcount"]
                chunks = []  # (tile_start, ntiles, bucket)
                t0 = 0
                for b in range(NBUCKET):
                    n = int(gc[b])
                    s = 0
                    while s < n:
                        c = min(SUP_CHUNK, n - s)
                        chunks.append((t0 + s, c, b))
                        s += c
                    t0 += n
                if ablate == "agonly":
                    chunks = []
                for (ts, ntl, b) in chunks:
                    nidx = ntl * 128
                    Ga = sup_p.tile([128, SUP_CHUNK * 128], F16, tag="Ga", name="Ga")
                    Gp = sup_p.tile([128, SUP_CHUNK * 128], F16, tag="Gp", name="Gp")
                    nc.gpsimd.dma_gather(
                        Ga[:, :ntl * 128].rearrange("p (c e) -> p c e", e=128),
                        a2_own[:], idx_sup_a_t[:, ts * 8:(ts + ntl) * 8],
                        nidx, nidx, 128, single_packet=(nidx <= 1024),
                        queue_num=(2 * ts) % 4)
                    bs = b * BUCKET
                    be = min(bs + BUCKET, N_AUTHOR)
                    nc.gpsimd.dma_gather(
                        Gp[:, :ntl * 128].rearrange("p (c e) -> p c e", e=128),
                        p2_full[bs:be, :], idx_sup_p_t[:, ts * 8:(ts + ntl) * 8],
                        nidx, nidx, 128, single_packet=(nidx <= 1024),
                        queue_num=(2 * ts + 1) % 4)
                    for t in range(ntl):
                        prod = sup_p.tile([128, 128], F32, tag="prod", name="prod")
                        nc.vector.tensor_tensor(
                            out=prod[:],
                            in0=Ga[:, t * 128:t * 128 + 128],
                            in1=Gp[:, t * 128:t * 128 + 128],
                            op=mybir.AluOpType.mult)
                        nc.vector.reduce_sum(
                            out=out_sb[:, ts + t:ts + t + 1], in_=prod[:],
                            axis=mybir.AxisListType.X)
            nc.sync.dma_start(out=out_sup[:], in_=out_sb[:])
    nc.compile()
    return nc


# ---------------------------------------------------------------- interface

_CACHE = {}


def _preprocess(inputs):
    xa = np.asarray(inputs["x_author"], np.float32).astype(np.float16)
    xp = np.asarray(inputs["x_paper"], np.float32)[:N_PAPER_ACT].astype(np.float16)
    ei = np.asarray(inputs["edge_index"], np.int64)
    ci = np.asarray(inputs["coauthor_edge_index"], np.int64)
    si = np.asarray(inputs["supervision_edge_index"], np.int64)

    idx_a2p, dl_a2p, deg_a2p, meta_a2p = _prep_direction(ei[0], ei[1])
    idx_p2a, dl_p2a, deg_p2a, meta_p2a = _prep_direction(ei[1], ei[0])
    idx_co, dl_co, deg_co, meta_co = _prep_direction(ci[0], ci[1])
    sup_packs, sup_gcount = _prep_sup(si[0], si[1])

    meta = {
        "a2p": meta_a2p, "p2a": meta_p2a, "co": meta_co,
        "ntiles_a2p": dl_a2p[0].shape[1],
        "ntiles_p2a": dl_p2a[0].shape[1],
        "ntiles_co": dl_co[0].shape[1],
        "sup_gcount": sup_gcount,
    }

    ws, bs = [], []
    for l in range(N_LAYERS):
        for nm in ["W_a2p", "W_pself", "W_p2a", "W_aself", "W_co"]:
            ws.append(np.asarray(inputs[nm], np.float32)[l].T.astype(np.float16))
        for pair in [("b_a2p", "b_pself"), ("b_p2a", "b_aself"),
                     ("b_co", None)]:
            r0 = np.asarray(inputs[pair[0]], np.float32)[l]
            r1 = (np.asarray(inputs[pair[1]], np.float32)[l]
                  if pair[1] else np.zeros(D, np.float32))
            bs.append(np.stack([r0, r1]).astype(np.float16))
    w_cat = np.concatenate(ws, axis=1)           # [128, 1280]
    # bias order: (l0: a2p, p2a, co), (l1: ...)
    bias_cat = np.concatenate(bs, axis=1)        # [2, 768]
    iota = np.broadcast_to(np.arange(512, dtype=np.float16), (128, 512)).copy()
    ident = np.eye(128, dtype=np.float16)

    in_maps = []
    for k in range(NCORES):
        in_maps.append({
            "author_t0": np.ascontiguousarray(xa),
            "paper_t0": np.ascontiguousarray(xp),
            "xaT0": xa[k * SHARD:(k + 1) * SHARD].T.copy(),
            "xpT0": xp[k * SHARD:(k + 1) * SHARD].T.copy(),
            "w_cat": w_cat, "bias_cat": bias_cat,
            "iota512": iota, "identity": ident,
            "idx_a2p": idx_a2p[k], "dl_a2p": dl_a2p[k], "deg_a2p": deg_a2p[k],
            "idx_p2a": idx_p2a[k], "dl_p2a": dl_p2a[k], "deg_p2a": deg_p2a[k],
            "idx_co": idx_co[k], "dl_co": dl_co[k], "deg_co": deg_co[k],
            "idx_sup_a": sup_packs[k][0],
            "idx_sup_p": sup_packs[k][1],
        })
    recon = [p[2] for p in sup_packs]
    return in_maps, meta, recon, si


def _postprocess(results, meta, recon):
    gc = meta["sup_gcount"]
    out = np.zeros(100000, np.float32)
    for k in range(NCORES):
        o = results[k]["out_sup"]          # [128, nsupt]
        t0 = 0
        for b in range(NBUCKET):
            pos = recon[k][b]
            n = len(pos)
            vals = o[:, t0:t0 + int(gc[b])].T.reshape(-1)[:n]
            out[pos] = vals
            t0 += int(gc[b])
    return out


def kernel(**inputs):
    in_maps, meta, recon, _si = _preprocess(inputs)
    key = "prog"
    if key not in _CACHE:
        _CACHE[key] = _build_program(meta)
    nc = _CACHE[key]
    res = run_bass_kernel_spmd(nc, in_maps, core_ids=list(range(NCORES)))
    return _postprocess(res.results, meta, recon)



# revision 16
# speedup vs baseline: 14.5209x; 1.7308x over previous
"""BipartiteGCN Trainium2 kernel (8 NeuronCores, Bass/Tile).

Strategy: shard message DESTINATIONS across cores (authors 12500/core, active
papers 12500/core; papers >= 100000 never influence the output since all paper
indices are drawn from [0, 100000)). Host sorts each core's edges by
destination into 512-dst superblocks and 32768-row source buckets, so the
device does:
  - dma_gather of 256B feature rows (int16 bucket-relative indices)
  - segment-sum via one-hot (iota is_equal) fp16 matmuls accumulated in PSUM
  - fused W_dir @ agg + W_self @ x_own + bias*deg via matmuls
  - PE transposes to write updated row-major shards, AllGather between phases
The six message phases are ordered p2a, a2p, co per layer so that every
AllGather overlaps the next (independent) message phase.
Supervision readout is sharded by author owner (author rows local, paper rows
gathered from the AllGathered table).
"""

import numpy as np

import concourse.bacc as bacc
import concourse.mybir as mybir
import concourse.tile as tile
from concourse.bass import AP
from concourse.bass_utils import run_bass_kernel_spmd

F32 = mybir.dt.float32
F32R = mybir.dt.float32r
F16 = mybir.dt.float16
I16 = mybir.dt.int16

NCORES = 8
D = 128
N_AUTHOR = 100000
N_PAPER_ACT = 100000          # active papers (indices ever referenced)
SHARD = N_AUTHOR // NCORES    # 12500 nodes per core (authors and papers)
SB = 512                      # superblock width (1 PSUM bank of fp32)
NSB = (SHARD + SB - 1) // SB  # 25 (24 full + one 212-wide)
MACRO = 4                     # superblocks per gather macro
NMACRO = (NSB + MACRO - 1) // MACRO
BUCKET = 32768                # int16 index range per source bucket
NBUCKET = (N_AUTHOR + BUCKET - 1) // BUCKET  # 4
WIN = 256                     # one-hot window width (non-first tiles)
PAD_DST = 5000.0              # out-of-window sentinel for pad edges
SUP_CHUNK = 8
EQ_BATCH = 4   # tiles per batched one-hot instruction
_UNUSED = 0                 # supervision gather chunk, in 128-pair tiles
N_LAYERS = 2


def _sb_width(sb):
    return min(SB, SHARD - sb * SB)


# ---------------------------------------------------------------- host prep

def _wrap_idx(idx):
    """Pack int index array (len multiple of 128) into the [128, n/16] int16
    dma_gather layout: index j at [j%16, j//16], replicated across the 8
    16-partition groups."""
    n = len(idx)
    w = np.zeros((128, n // 16), np.int16)
    base = idx.astype(np.int16).reshape(n // 16, 16).T  # [16, n/16]
    for g in range(8):
        w[16 * g:16 * g + 16, :] = base
    return w


def _build_tiles_one_core(src, dst_local):
    """Split one core's edges of one direction into gather tiles.

    Tiles are grouped by (superblock, source bucket); every tile targets the
    full 512-wide PSUM window of its superblock. Returns nt[sb][bucket]
    counts and dict (sb,b) -> list of (src_rel128, off128) tile contents."""
    sb_id = dst_local // SB
    off = dst_local - sb_id * SB
    bucket = src // BUCKET
    rel = src - bucket * BUCKET
    tiles = {}
    nt = np.zeros((NSB, NBUCKET), np.int64)
    order = np.lexsort((off, bucket, sb_id))
    sb_s, b_s = sb_id[order], bucket[order]
    off_s, rel_s = off[order], rel[order]
    key = sb_s * NBUCKET + b_s
    bounds = np.flatnonzero(np.diff(key)) + 1
    starts = np.concatenate(([0], bounds))
    ends = np.concatenate((bounds, [len(key)]))
    for s, e in zip(starts, ends):
        sb, b = int(sb_s[s]), int(b_s[s])
        o = off_s[s:e]
        r = rel_s[s:e]
        group = [(r[i:i + 128], o[i:i + 128]) for i in range(0, e - s, 128)]
        tiles[(sb, b)] = group
        nt[sb, b] = len(group)
    return nt, tiles


def _emit_direction(all_tiles, global_nt):
    """Produce, for one core, the packed idx stream / dstloc array / tile meta
    given equalized per-(sb,bucket) tile counts global_nt.

    Returns (idx_wrapped [128, NTOT/16], dstloc [128, NTILES],
             meta list over (macro, bucket) -> list of (sb, first, last))."""
    idx_stream = []
    dstloc_cols = []
    meta = []          # per (m, b): list of tile tuples
    first_seen = set()
    # per-sb last tile position in bucket-major order
    last_pos = {}
    for sb in range(NSB):
        tot = int(global_nt[sb].sum())
        assert tot > 0
        c = 0
        for b in range(NBUCKET):
            for t in range(int(global_nt[sb, b])):
                c += 1
                if c == tot:
                    last_pos[sb] = (b, t)
    for m in range(NMACRO):
        sbs = range(m * MACRO, min((m + 1) * MACRO, NSB))
        for b in range(NBUCKET):
            tl = []
            for sb in sbs:
                group = all_tiles.get((sb, b), [])
                for t in range(int(global_nt[sb, b])):
                    if t < len(group):
                        r, o = group[t]
                    else:
                        r = np.zeros(0, np.int64)
                        o = np.zeros(0, np.int64)
                    n = len(r)
                    first = sb not in first_seen
                    if first:
                        first_seen.add(sb)
                    last = last_pos[sb] == (b, t)
                    src128 = np.zeros(128, np.int64)
                    dl128 = np.full(128, PAD_DST, np.float16)
                    src128[:n] = r
                    dl128[:n] = o[:n]
                    idx_stream.append(src128)
                    dstloc_cols.append(dl128)
                    tl.append((sb, first, last))
            meta.append(tl)
    ntiles = len(idx_stream)
    idx_flat = np.concatenate(idx_stream) if ntiles else np.zeros(0, np.int64)
    dstloc = (np.stack(dstloc_cols, axis=1) if ntiles
              else np.zeros((128, 0), np.float16))
    return _wrap_idx(idx_flat), dstloc.astype(np.float16), meta


def _prep_direction(src_all, dst_all, ncores=NCORES):
    """Full host prep of one message direction. src_all/dst_all are global
    edge arrays; dst determines owning core. Returns per-core packed arrays
    plus the (core-independent) meta."""
    owner = dst_all // SHARD
    per_core = []
    nts = []
    for k in range(ncores):
        m = owner == k
        nt, tiles = _build_tiles_one_core(src_all[m], dst_all[m] - k * SHARD)
        nts.append(nt)
        per_core.append(tiles)
    global_nt = np.maximum.reduce(nts)
    global_nt[:, 0] = np.maximum(global_nt[:, 0], 1)  # sb needs a first tile
    idxs, dstlocs, metas = [], [], []
    for k in range(ncores):
        iw, dl, meta = _emit_direction(per_core[k], global_nt)
        idxs.append(iw)
        dstlocs.append(dl)
        metas.append(meta)
    # degrees per destination
    degs = []
    for k in range(ncores):
        m = owner == k
        deg = np.bincount(dst_all[m] - k * SHARD, minlength=SHARD)
        degs.append(np.stack([deg, np.ones(SHARD)]).astype(np.float16))
    return idxs, dstlocs, degs, metas[0]


def _prep_sup(sup_a, sup_p, ncores=NCORES):
    """Supervision pairs sharded by author owner; sorted by paper bucket.
    Returns per-core (a_idx_wrapped, p_idx_wrapped, positions), per-bucket
    tile counts (core-uniform)."""
    owner = sup_a // SHARD
    per_core = []
    counts = np.zeros((ncores, NBUCKET), np.int64)
    for k in range(ncores):
        m = np.flatnonzero(owner == k)
        a = sup_a[m] - k * SHARD
        p = sup_p[m]
        b = p // BUCKET
        order = np.argsort(b, kind="stable")
        per_core.append((a[order], p[order], b[order], m[order]))
        for bb in range(NBUCKET):
            counts[k, bb] = int(np.ceil((b == bb).sum() / 128.0))
    gcount = np.maximum(counts.max(axis=0), 1)
    packs = []
    for k in range(ncores):
        a, p, b, pos = per_core[k]
        a_st, p_st, pos_st = [], [], []
        for bb in range(NBUCKET):
            m = b == bb
            ab, pb, posb = a[m], p[m] - bb * BUCKET, pos[m]
            n = int(gcount[bb]) * 128
            a128 = np.zeros(n, np.int64)
            p128 = np.zeros(n, np.int64)
            a128[:len(ab)] = ab
            p128[:len(pb)] = pb
            a_st.append(a128)
            p_st.append(p128)
            pos_st.append(posb)
        packs.append((_wrap_idx(np.concatenate(a_st)),
                      _wrap_idx(np.concatenate(p_st)),
                      pos_st))
    return packs, gcount


# ------------------------------------------------------------- program build

def _build_program(meta, null=False, reps=1, ablate=None):
    """meta: dict with keys a2p/p2a/co -> per-(macro,bucket) tile meta,
    ntiles per direction, sup gcount."""
    nc = bacc.Bacc("TRN2", target_bir_lowering=False, debug=False,
                   enable_asserts=False, num_devices=NCORES,
                   num_swdge_queues=4)
    dt_in = {}

    def din(name, shape, dt=F16):
        dt_in[name] = nc.dram_tensor(name, shape, dt, kind="ExternalInput").ap()
        return dt_in[name]

    author_t0 = din("author_t0", [N_AUTHOR, D])
    paper_t0 = din("paper_t0", [N_PAPER_ACT, D])
    xaT0 = din("xaT0", [128, SHARD])
    xpT0 = din("xpT0", [128, SHARD])
    w_cat = din("w_cat", [128, 128 * 10])
    bias_cat = din("bias_cat", [2, 128 * 6])
    iota_in = din("iota512", [128, 512], F16)
    ident_in = din("identity", [128, 128])
    dirs = ["a2p", "p2a", "co"]
    idx_in, dl_in, deg_in = {}, {}, {}
    for d in dirs:
        nt = meta[f"ntiles_{d}"]
        idx_in[d] = din(f"idx_{d}", [128, nt * 8], I16)
        dl_in[d] = din(f"dl_{d}", [128, nt], F16)
        deg_in[d] = din(f"deg_{d}", [2, SHARD])
    nsup = int(meta["sup_gcount"].sum()) * 128
    idx_sup_a = din("idx_sup_a", [128, nsup // 16], I16)
    idx_sup_p = din("idx_sup_p", [128, nsup // 16], I16)
    nsupt = nsup // 128
    out_sup = nc.dram_tensor("out_sup", [128, nsupt], F32,
                             kind="ExternalOutput").ap()

    # max gather size (tiles) over (macro, bucket) for SBUF sizing
    gmax = 1
    for d in dirs:
        for tl in meta[d]:
            gmax = max(gmax, len(tl))
    # max idx columns per macro
    idx_cols_max = 16
    for d in dirs:
        mm = meta[d]
        for m in range(NMACRO):
            c = sum(len(mm[m * NBUCKET + b]) for b in range(NBUCKET)) * 8
            idx_cols_max = max(idx_cols_max, c)

    if null:
        with tile.TileContext(nc) as tc:
            with tc.tile_pool(name="nsb", bufs=1) as sbp:
                z = sbp.tile([128, nsupt], F32, name="z")
                t0 = sbp.tile([128, 128], F16, name="t0")
                nc.sync.dma_start(out=t0[:], in_=author_t0[0:128, :])
                nc.vector.memset(z[:], 0.0)
                nc.sync.dma_start(out=out_sup[:], in_=z[:])
        nc.compile()
        return nc

    with tile.TileContext(nc) as tc:
        with tc.tile_pool(name="persist", bufs=1) as pp, \
             tc.tile_pool(name="gat", bufs=2) as gp, \
             tc.tile_pool(name="oneh", bufs=6) as sp, \
             tc.tile_pool(name="stageb", bufs=3) as bp, \
             tc.tile_pool(name="degp", bufs=4) as dgp, \
             tc.tile_pool(name="idxp", bufs=2) as ixp, \
             tc.tile_pool(name="supp", bufs=2) as sup_p, \
             tc.tile_pool(name="psA", bufs=5, space="PSUM") as psA, \
             tc.tile_pool(name="psB", bufs=2, space="PSUM") as psB, \
             tc.tile_pool(name="psT", bufs=1, space="PSUM") as psT, \
             tc.tile_pool(name="dram", bufs=1, space="DRAM") as drp:

            # ---- persistent state ----
            xaT = pp.tile([128, SHARD], F16, name="xaT")
            xpT = pp.tile([128, SHARD], F16, name="xpT")
            iota = pp.tile([128, 512], F16, name="iota")
            ident = pp.tile([128, 128], F16, name="ident")
            w_t = pp.tile([128, 128 * 10], F16, name="w_t")
            bias_t = pp.tile([2, 128 * 6], F16, name="bias_t")
            dl_t = {d: pp.tile([128, meta[f"ntiles_{d}"]], F16, name=f"dl_{d}")
                    for d in dirs}
            out_sb = pp.tile([128, nsupt], F32, name="out_sb")
            if ablate == "agonly":
                nc.vector.memset(out_sb[:], 0.0)

            nc.sync.dma_start(out=xaT[:], in_=xaT0[:])
            nc.sync.dma_start(out=xpT[:], in_=xpT0[:])
            nc.sync.dma_start(out=iota[:], in_=iota_in[:])
            nc.sync.dma_start(out=ident[:], in_=ident_in[:])
            nc.sync.dma_start(out=w_t[:], in_=w_cat[:])
            nc.sync.dma_start(out=bias_t[:], in_=bias_cat[:])
            for d in dirs:
                nc.sync.dma_start(out=dl_t[d][:], in_=dl_in[d][:])
            idx_sup_a_t = pp.tile([128, nsup // 16], I16, name="supa")
            idx_sup_p_t = pp.tile([128, nsup // 16], I16, name="supb")
            nc.sync.dma_start(out=idx_sup_a_t[:], in_=idx_sup_a[:])
            nc.sync.dma_start(out=idx_sup_p_t[:], in_=idx_sup_p[:])

            # ---- internal DRAM tables ----
            def dram_full(name):
                return drp.tile([N_AUTHOR, D], F16, addr_space="Shared",
                                name=name)

            def dram_own(name):
                return drp.tile([SHARD, D], F16, name=name)

            a1a_own = dram_own("a1a_o")
            a1_own = dram_own("a1_o")
            p1_own = dram_own("p1_o")
            a2a_own = dram_own("a2a_o")
            p2_own = dram_own("p2_o")
            a2_own = dram_own("a2_o")

            def w_slice(l, slot):
                o = (l * 5 + slot) * 128
                return w_t[:, o:o + 128]

            def bias_slice(l, ph):
                o = (l * 3 + ph) * 128
                return bias_t[:, o:o + 128]

            def process_direction(d, l, ph, src_tbl, xown, wdir, wself,
                                  biasp, own_out, co_mode=False):
                """One direction of one layer: stage A (gather+one-hot
                matmuls), stage B per superblock, row-major writeback."""
                if ablate == "agonly":
                    return
                mm = meta[d]
                nt_dir = meta[f"ntiles_{d}"]
                deg_d = deg_in[d]
                tile_col = 0
                psum_of_sb = {}
                left_of_sb = {sb: 0 for sb in range(NSB)}
                for tl in mm:
                    for (sb, _f, _l) in tl:
                        left_of_sb[sb] += 1
                # idx stream column offset per macro
                col_off = 0
                for m in range(NMACRO):
                    cols = sum(len(mm[m * NBUCKET + b]) for b in range(NBUCKET)) * 8
                    if cols == 0:
                        continue
                    idx_t = ixp.tile([128, idx_cols_max], I16, tag="idx", name="idxt")
                    nc.sync.dma_start(
                        out=idx_t[:, :cols],
                        in_=idx_in[d][:, col_off:col_off + cols])
                    mac_off = 0
                    for b in range(NBUCKET):
                        tl = mm[m * NBUCKET + b]
                        ntl = len(tl)
                        if ntl == 0:
                            continue
                        nidx = ntl * 128
                        bs = b * BUCKET
                        be = min(bs + BUCKET, N_AUTHOR)
                        G = gp.tile([128, gmax * 128], F16, tag="G", name="G")
                        nc.gpsimd.dma_gather(
                            G[:, :ntl * 128].rearrange(
                                "p (c e) -> p c e", e=128),
                            src_tbl[bs:be, :],
                            idx_t[:, mac_off:mac_off + ntl * 8],
                            nidx, nidx, 128,
                            single_packet=(nidx <= 1024), queue_num=b)
                        mac_off += ntl * 8
                        # batched one-hot: S[p, t*512+j] =
                        #   (iota[p, j] == dl[p, tile_col+t]) for runs of
                        #   EQ_BATCH tiles in one stride-0 tensor_tensor
                        S_of = {}
                        for r0 in range(0, ntl, EQ_BATCH):
                            n = min(EQ_BATCH, ntl - r0)
                            S = sp.tile([128, EQ_BATCH * 512], F16,
                                        tag="S", name="S")
                            iap = iota[:, :512]
                            dap = dl_t[d][:, tile_col + r0:tile_col + r0 + n]
                            oap = S[:, :n * 512]
                            nc.vector.tensor_tensor(
                                out=AP(oap.tensor, oap.offset,
                                       [oap.ap[0], [512, n], [1, 512]]),
                                in0=AP(iap.tensor, iap.offset,
                                       [iap.ap[0], [0, n], iap.ap[1]]),
                                in1=AP(dap.tensor, dap.offset,
                                       [dap.ap[0], dap.ap[1], [0, 512]]),
                                op=mybir.AluOpType.is_equal)
                            for t in range(n):
                                S_of[r0 + t] = (S, t * 512)
                        for ti, (sb, first, _last) in enumerate(tl):
                            if sb not in psum_of_sb:
                                psum_of_sb[sb] = psA.tile(
                                    [128, 512], F32, tag="agg", name="agg")
                            pa = psum_of_sb[sb]
                            S, so = S_of[ti]
                            left_of_sb[sb] -= 1
                            nc.tensor.matmul(
                                out=pa[:, :],
                                lhsT=G[:, ti * 128:ti * 128 + 128],
                                rhs=S[:, so:so + 512],
                                start=first, stop=(left_of_sb[sb] == 0))
                            tile_col += 1
                    # stage B for completed superblocks of this macro
                    for sb in range(m * MACRO, min((m + 1) * MACRO, NSB)):
                        if sb not in psum_of_sb:
                            continue
                        wdt = _sb_width(sb)
                        pa = psum_of_sb.pop(sb)
                        agg_sb = bp.tile([128, 512], F16, tag="aggsb", name="aggsb")
                        nc.scalar.activation(
                            out=agg_sb[:, :wdt], in_=pa[:, :wdt],
                            func=mybir.ActivationFunctionType.Copy)
                        deg_t = dgp.tile([2, 512], F16, tag="deg", name="degt")
                        nc.sync.dma_start(
                            out=deg_t[:, :wdt],
                            in_=deg_d[:, sb * SB:sb * SB + wdt])
                        pb = psB.tile([128, 512], F32, tag="out", name="pb")
                        nc.tensor.matmul(out=pb[:, :wdt], lhsT=wdir,
                                         rhs=agg_sb[:, :wdt],
                                         start=True, stop=False)
                        if not co_mode:
                            nc.tensor.matmul(
                                out=pb[:, :wdt], lhsT=wself,
                                rhs=xown[:, sb * SB:sb * SB + wdt],
                                start=False, stop=False)
                        nc.tensor.matmul(out=pb[:, :wdt], lhsT=biasp,
                                         rhs=deg_t[:2, :wdt],
                                         start=False, stop=True)
                        if co_mode:
                            nc.vector.tensor_tensor(
                                out=xown[:, sb * SB:sb * SB + wdt],
                                in0=pb[:, :wdt],
                                in1=xown[:, sb * SB:sb * SB + wdt],
                                op=mybir.AluOpType.add)
                        else:
                            nc.scalar.activation(
                                out=xown[:, sb * SB:sb * SB + wdt],
                                in_=pb[:, :wdt],
                                func=mybir.ActivationFunctionType.Copy)
                        # transpose to row-major and write the shard slice
                        pt = psT.tile([128, 512], F16, tag="tr", name="pt")
                        nchunk = (wdt + 127) // 128
                        for j in range(nchunk):
                            cw = min(128, wdt - j * 128)
                            nc.tensor.matmul(
                                out=pt[:cw, j * 128:j * 128 + 128],
                                lhsT=xown[:, sb * SB + j * 128:
                                          sb * SB + j * 128 + cw],
                                rhs=ident[:],
                                is_transpose=True,
                                start=(j == 0), stop=(j == nchunk - 1))
                        rm = bp.tile([128, 512], F16, tag="rm", name="rm")
                        nc.scalar.activation(
                            out=rm[:, :nchunk * 128], in_=pt[:, :nchunk * 128],
                            func=mybir.ActivationFunctionType.Copy)
                        for j in range(nchunk):
                            cw = min(128, wdt - j * 128)
                            nc.sync.dma_start(
                                out=own_out[sb * SB + j * 128:
                                            sb * SB + j * 128 + cw, 0:128],
                                in_=rm[:cw, j * 128:j * 128 + 128])
                    col_off += cols

            def allgather(own, full):
                if ablate == "noag":
                    return
                nc.gpsimd.collective_compute(
                    "AllGather", mybir.AluOpType.bypass,
                    replica_groups=[list(range(NCORES))],
                    ins=[own[:]], outs=[full[:]])

            # ---------------- pipeline (repeated for timing) ----------
            for _rep in range(reps):
                # Shared tiles may only have one (collective) writer; fresh
                # AG outputs per repetition
                a1a_full = dram_full(f"a1a_f{_rep}")
                a1_full = dram_full(f"a1_f{_rep}")
                p1_full = dram_full(f"p1_f{_rep}")
                a2a_full = dram_full(f"a2a_f{_rep}")
                p2_full = dram_full(f"p2_f{_rep}")
                # Order pairs every AllGather with an independent message
                # phase so the collective transfer hides under compute:
                #   p2a(0); AG(a1a) || a2p(0); AG(p1) || co(0);
                #   AG(a1) || p2a(1); AG(a2a) || a2p(1); AG(p2) || co(1); sup
                process_direction("p2a", 0, 1, paper_t0, xaT,
                                  w_slice(0, 2), w_slice(0, 3),
                                  bias_slice(0, 1), a1a_own)
                process_direction("a2p", 0, 0, author_t0, xpT,
                                  w_slice(0, 0), w_slice(0, 1),
                                  bias_slice(0, 0), p1_own)
                allgather(a1a_own, a1a_full)
                process_direction("co", 0, 2, a1a_full[:], xaT,
                                  w_slice(0, 4), None, bias_slice(0, 2),
                                  a1_own, co_mode=True)
                allgather(p1_own, p1_full)
                process_direction("p2a", 1, 1, p1_full[:], xaT,
                                  w_slice(1, 2), w_slice(1, 3),
                                  bias_slice(1, 1), a2a_own)
                allgather(a1_own, a1_full)
                process_direction("a2p", 1, 0, a1_full[:], xpT,
                                  w_slice(1, 0), w_slice(1, 1),
                                  bias_slice(1, 0), p2_own)
                allgather(a2a_own, a2a_full)
                process_direction("co", 1, 2, a2a_full[:], xaT,
                                  w_slice(1, 4), None, bias_slice(1, 2),
                                  a2_own, co_mode=True)
                allgather(p2_own, p2_full)

                # ---------------- supervision readout ----------------
                gc = meta["sup_gcount"]
                chunks = []  # (tile_start, ntiles, bucket)
                t0 = 0
                for b in range(NBUCKET):
                    n = int(gc[b])
                    s = 0
                    while s < n:
                        c = min(SUP_CHUNK, n - s)
                        chunks.append((t0 + s, c, b))
                        s += c
                    t0 += n
                if ablate == "agonly":
                    chunks = []
                for (ts, ntl, b) in chunks:
                    nidx = ntl * 128
                    Ga = sup_p.tile([128, SUP_CHUNK * 128], F16, tag="Ga", name="Ga")
                    Gp = sup_p.tile([128, SUP_CHUNK * 128], F16, tag="Gp", name="Gp")
                    nc.gpsimd.dma_gather(
                        Ga[:, :ntl * 128].rearrange("p (c e) -> p c e", e=128),
                        a2_own[:], idx_sup_a_t[:, ts * 8:(ts + ntl) * 8],
                        nidx, nidx, 128, single_packet=(nidx <= 1024),
                        queue_num=(2 * ts) % 4)
                    bs = b * BUCKET
                    be = min(bs + BUCKET, N_AUTHOR)
                    nc.gpsimd.dma_gather(
                        Gp[:, :ntl * 128].rearrange("p (c e) -> p c e", e=128),
                        p2_full[bs:be, :], idx_sup_p_t[:, ts * 8:(ts + ntl) * 8],
                        nidx, nidx, 128, single_packet=(nidx <= 1024),
                        queue_num=(2 * ts + 1) % 4)
                    for t in range(ntl):
                        prod = sup_p.tile([128, 128], F32, tag="prod", name="prod")
                        nc.vector.tensor_tensor(
                            out=prod[:],
                            in0=Ga[:, t * 128:t * 128 + 128],
                            in1=Gp[:, t * 128:t * 128 + 128],
                            op=mybir.AluOpType.mult)
                        nc.vector.reduce_sum(
                            out=out_sb[:, ts + t:ts + t + 1], in_=prod[:],
                            axis=mybir.AxisListType.X)
            nc.sync.dma_start(out=out_sup[:], in_=out_sb[:])
    nc.compile()
    return nc


# ---------------------------------------------------------------- interface

_CACHE = {}


def _preprocess(inputs):
    xa = np.asarray(inputs["x_author"], np.float32).astype(np.float16)
    xp = np.asarray(inputs["x_paper"], np.float32)[:N_PAPER_ACT].astype(np.float16)
    ei = np.asarray(inputs["edge_index"], np.int64)
    ci = np.asarray(inputs["coauthor_edge_index"], np.int64)
    si = np.asarray(inputs["supervision_edge_index"], np.int64)

    idx_a2p, dl_a2p, deg_a2p, meta_a2p = _prep_direction(ei[0], ei[1])
    idx_p2a, dl_p2a, deg_p2a, meta_p2a = _prep_direction(ei[1], ei[0])
    idx_co, dl_co, deg_co, meta_co = _prep_direction(ci[0], ci[1])
    sup_packs, sup_gcount = _prep_sup(si[0], si[1])

    meta = {
        "a2p": meta_a2p, "p2a": meta_p2a, "co": meta_co,
        "ntiles_a2p": dl_a2p[0].shape[1],
        "ntiles_p2a": dl_p2a[0].shape[1],
        "ntiles_co": dl_co[0].shape[1],
        "sup_gcount": sup_gcount,
    }

    ws, bs = [], []
    for l in range(N_LAYERS):
        for nm in ["W_a2p", "W_pself", "W_p2a", "W_aself", "W_co"]:
            ws.append(np.asarray(inputs[nm], np.float32)[l].T.astype(np.float16))
        for pair in [("b_a2p", "b_pself"), ("b_p2a", "b_aself"),
                     ("b_co", None)]:
            r0 = np.asarray(inputs[pair[0]], np.float32)[l]
            r1 = (np.asarray(inputs[pair[1]], np.float32)[l]
                  if pair[1] else np.zeros(D, np.float32))
            bs.append(np.stack([r0, r1]).astype(np.float16))
    w_cat = np.concatenate(ws, axis=1)           # [128, 1280]
    # bias order: (l0: a2p, p2a, co), (l1: ...)
    bias_cat = np.concatenate(bs, axis=1)        # [2, 768]
    iota = np.broadcast_to(np.arange(512, dtype=np.float16), (128, 512)).copy()
    ident = np.eye(128, dtype=np.float16)

    in_maps = []
    for k in range(NCORES):
        in_maps.append({
            "author_t0": np.ascontiguousarray(xa),
            "paper_t0": np.ascontiguousarray(xp),
            "xaT0": xa[k * SHARD:(k + 1) * SHARD].T.copy(),
            "xpT0": xp[k * SHARD:(k + 1) * SHARD].T.copy(),
            "w_cat": w_cat, "bias_cat": bias_cat,
            "iota512": iota, "identity": ident,
            "idx_a2p": idx_a2p[k], "dl_a2p": dl_a2p[k], "deg_a2p": deg_a2p[k],
            "idx_p2a": idx_p2a[k], "dl_p2a": dl_p2a[k], "deg_p2a": deg_p2a[k],
            "idx_co": idx_co[k], "dl_co": dl_co[k], "deg_co": deg_co[k],
            "idx_sup_a": sup_packs[k][0],
            "idx_sup_p": sup_packs[k][1],
        })
    recon = [p[2] for p in sup_packs]
    return in_maps, meta, recon, si


def _postprocess(results, meta, recon):
    gc = meta["sup_gcount"]
    out = np.zeros(100000, np.float32)
    for k in range(NCORES):
        o = results[k]["out_sup"]          # [128, nsupt]
        t0 = 0
        for b in range(NBUCKET):
            pos = recon[k][b]
            n = len(pos)
            vals = o[:, t0:t0 + int(gc[b])].T.reshape(-1)[:n]
            out[pos] = vals
            t0 += int(gc[b])
    return out


def kernel(**inputs):
    in_maps, meta, recon, _si = _preprocess(inputs)
    key = "prog"
    if key not in _CACHE:
        _CACHE[key] = _build_program(meta)
    nc = _CACHE[key]
    res = run_bass_kernel_spmd(nc, in_maps, core_ids=list(range(NCORES)))
    return _postprocess(res.results, meta, recon)

